# revision 15
# baseline (speedup 1.0000x reference)
"""Multi-Scale Deformable Attention (DigitDETR encoder layer) on 8 TRN2 cores.

Sharding: 16 (batch, head) pairs over 8 cores -> each core handles one batch
and two consecutive heads (data-parallel over B, tensor-parallel over H).
Each core computes a partial output  msda(b, h0..h1) @ W_out[h-rows]; the host
sums the 4 partials per batch and adds b_out during unsharding.

Per-core device pipeline (Tile framework):
  P1  value projection (PE) -> v[h] in DRAM -> column-major x-duplicated
      "patch table" per head: row r = pixel (x, y), content = [v(y,x), v(y,x+1)]
      (64 f32).  Rows r, r+1 are y-neighbors, so one 512B read at row r
      yields the full 2x2 bilinear patch (elem_size=128, elem_step=64).
  P2  fused projection matmul (query_T | ref*size-1.0 | ones) @ W_aug gives
      px' (= px-0.5), py', attn logits; softmax; floor via +/-2^23 magic;
      clip; slot weights relu(1-|d±.5|) with border masks; idx f32.
  P3  gather via dma_gather with sample order i = lp*128+q so G lands
      q-partitioned [q, lp, 4, 32]; idx wrap [q%16, lp*8+q//16] built by a
      masked-replicate matmul (R = idx*maskC; psw = S128 @ R -> 8 replicas
      for sim partitions 0-15 + all 4 SWDGE queue core pairs); gathers
      round-robin queues 0-3 so 4 Q7 pairs generate descriptors in
      parallel; DVE product with w4 read straight from P2 (no DRAM
      bounce); 2 DVE reduces (slots, then lp) -> msda[q, 64] in SBUF.
  P4  PE-transpose msda, out-projection matmul -> rows.
"""

import ml_dtypes
import numpy as np

import concourse.bass as bass
import concourse.bacc as bacc
import concourse.mybir as mybir
import concourse.tile as tile
from concourse.bass_utils import run_bass_kernel_spmd
from concourse.masks import make_identity

F32 = mybir.dt.float32
FP16 = mybir.dt.float16
BF16 = mybir.dt.bfloat16
I16 = mybir.dt.int16
AX = mybir.AxisListType
OP = mybir.AluOpType
ACT = mybir.ActivationFunctionType

# ---- static problem config ----
SPATIAL = ((76, 114), (38, 57), (19, 29), (10, 15))  # (lh, lw)
HWS = [h * w for h, w in SPATIAL]
STARTS = [0, 8664, 10830, 11381]
NV = 11531
B, H, L, P, DM, HD = 2, 8, 4, 4, 256, 32
NQ = NV
NT = 91
NQP = NT * 128          # 11648
KA = DM + 2 * L * P + 1  # 289
KV = DM + 1              # 257
MAGIC = 12582912.0       # 1.5 * 2^23
CHUNK = 8

N_CORES = 8


def _kchunks(k):
    out, o = [], 0
    while o < k:
        kk = min(128, k - o)
        out.append((o, kk))
        o += kk
    return out


KA_CH = _kchunks(KA)
KV_CH = _kchunks(KV)


def build_module(reps=1):
    nc = bacc.Bacc("TRN2", target_bir_lowering=False, debug=False,
                   enable_asserts=False, num_devices=N_CORES,
                   num_swdge_queues=4)

    qaug = nc.dram_tensor("qaug", [KA, NQP], F32, kind="ExternalInput").ap()
    vaug = nc.dram_tensor("vaug", [KV, NQP], BF16, kind="ExternalInput").ap()
    waug = nc.dram_tensor("waug", [KA, 96], F32, kind="ExternalInput").ap()
    wvaug = nc.dram_tensor("wvaug", [KV, 64], BF16, kind="ExternalInput").ap()
    wout = nc.dram_tensor("wout", [64, 256], F32, kind="ExternalInput").ap()
    # rows: 0=lw, 1=lw-1, 2=lh-1, 3=start, 4=lw-1.5, 5=lh-1.5
    consts = nc.dram_tensor("consts", [128, 6, 16], F32, kind="ExternalInput").ap()
    # wrapc[:, :128] = S128[q, m] = (q%16 == m%16); [:, 128:136] = maskC[q, c]
    # = (q//16 == c)
    wrapc = nc.dram_tensor("wrapc", [128, 136], F32, kind="ExternalInput").ap()
    outp = nc.dram_tensor("outp", [NQP, 256], F32, kind="ExternalOutput").ap()

    with tile.TileContext(nc) as tc:
        with (
            tc.tile_pool(name="dram", bufs=1, space="DRAM") as dpool,
            tc.tile_pool(name="const", bufs=1) as cpool,
            tc.tile_pool(name="stat", bufs=6) as spool,
            tc.tile_pool(name="work", bufs=2) as wpool,
            tc.tile_pool(name="gbuf", bufs=6) as gpool,
            tc.tile_pool(name="small", bufs=3) as mpool,
            tc.tile_pool(name="idx", bufs=18) as ipool,
            tc.tile_pool(name="psA", bufs=2, space="PSUM") as psA,
            tc.tile_pool(name="psT", bufs=2, space="PSUM") as psT,
            tc.tile_pool(name="psB", bufs=2, space="PSUM") as psB,
        ):
            # ---- resident constants ----
            ident = cpool.tile([128, 128], F32, tag="ident")
            make_identity(nc, ident[:])
            const_sb = cpool.tile([128, 6, 16], F32, tag="consts")
            nc.sync.dma_start(out=const_sb[:], in_=consts)
            lw_c = const_sb[:, 0, :]
            lwm1_c = const_sb[:, 1, :]
            lhm1_c = const_sb[:, 2, :]
            start_c = const_sb[:, 3, :]
            lwm15_c = const_sb[:, 4, :]
            lhm15_c = const_sb[:, 5, :]

            wrap_sb = cpool.tile([128, 136], F32, tag="wrapc")
            nc.sync.dma_start(out=wrap_sb[:], in_=wrapc)
            s128_c = wrap_sb[:, 0:128]
            maskc_c = wrap_sb[:, 128:136]

            wa_sb = []
            for i, (o, kk) in enumerate(KA_CH):
                t = cpool.tile([kk, 96], F32, tag=f"wa{i}", name=f"wa{i}")
                nc.sync.dma_start(out=t[:], in_=waug[o:o + kk, :])
                wa_sb.append(t)
            wv_sb = []
            for i, (o, kk) in enumerate(KV_CH):
                t = cpool.tile([kk, 64], BF16, tag=f"wv{i}", name=f"wv{i}")
                nc.sync.dma_start(out=t[:], in_=wvaug[o:o + kk, :])
                wv_sb.append(t)
            wout_sb = cpool.tile([64, 256], F32, tag="wout")
            nc.sync.dma_start(out=wout_sb[:], in_=wout)

            bp05 = cpool.tile([128, 1], F32, tag="bp05")
            nc.vector.memset(bp05[:], 0.5)
            bm05 = cpool.tile([128, 1], F32, tag="bm05")
            nc.vector.memset(bm05[:], -0.5)

            ni_reg = nc.gpsimd.to_reg(2048)

            # ---- DRAM scratch ----
            vtab = [dpool.tile([NQP, 32], FP16, tag=f"vtab{h}", name=f"vtab{h}")
                    for h in range(2)]
            tab = [dpool.tile([NQP, 128], FP16, tag=f"tab{h}", name=f"tab{h}")
                   for h in range(2)]
            tabv = [tab[h][:] for h in range(2)]

            # ---- pipelined P2/P3/P4: emission order = engine-queue order,
            # so chunk c+1's projections/indices are emitted before chunk c's
            # gather consumers to keep every engine fed while Pool gathers. ----
            i16s = {}    # (chunk_t0, i, h) -> i16 tile
            w4hs = {}    # chunk_t0 -> [w4h_h0, w4h_h1]

            def emit_p2(t0):
                nt = min(CHUNK, NT - t0)
                proj = wpool.tile([128, CHUNK, 96], F32, tag="proj")
                qas = []
                for j, (o, kk) in enumerate(KA_CH):
                    qa = spool.tile([128, CHUNK * 128], F32, tag="ld",
                                    name=f"qab{j}")
                    nc.scalar.dma_start(
                        out=qa[:kk, :nt * 128],
                        in_=qaug[o:o + kk, t0 * 128:(t0 + nt) * 128])
                    qas.append(qa)
                for i in range(nt):
                    psp = psA.tile([128, 96], F32, tag="ps_a")
                    for j, (o, kk) in enumerate(KA_CH):
                        nc.tensor.matmul(
                            out=psp[:], lhsT=qas[j][:kk, i * 128:(i + 1) * 128],
                            rhs=wa_sb[j][:],
                            start=(j == 0), stop=(j == len(KA_CH) - 1))
                    nc.scalar.copy(out=proj[:, i, :], in_=psp[:])

                w4hc = [None, None]
                for h in range(2):
                    c0 = h * 48
                    px = proj[:, :nt, c0 + 0:c0 + 16]
                    py = proj[:, :nt, c0 + 16:c0 + 32]
                    att = proj[:, :nt, c0 + 32:c0 + 48]

                    def bc(c16):
                        return c16.unsqueeze(1).to_broadcast([128, nt, 16])

                    # softmax over 16 (l,p)
                    e = wpool.tile([128, CHUNK, 16], F32, tag=f"e{h}")
                    nc.scalar.activation(out=e[:, :nt, :], in_=att, func=ACT.Exp)
                    ssum = mpool.tile([128, CHUNK], F32, tag=f"ss{h}")
                    nc.vector.tensor_reduce(out=ssum[:, :nt], in_=e[:, :nt, :],
                                            axis=AX.X, op=OP.add)
                    rinv = mpool.tile([128, CHUNK], F32, tag=f"ri{h}")
                    nc.vector.reciprocal(out=rinv[:, :nt], in_=ssum[:, :nt])
                    nc.vector.tensor_tensor(
                        out=e[:, :nt, :], in0=e[:, :nt, :],
                        in1=rinv[:, :nt].unsqueeze(2).to_broadcast([128, nt, 16]),
                        op=OP.mult)

                    # floor/clip
                    x0f = wpool.tile([128, CHUNK, 16], F32, tag=f"x0{h}")
                    y0f = wpool.tile([128, CHUNK, 16], F32, tag=f"y0{h}")
                    nc.vector.tensor_scalar(out=x0f[:, :nt, :], in0=px,
                                            scalar1=MAGIC, scalar2=MAGIC,
                                            op0=OP.add, op1=OP.subtract)
                    nc.vector.tensor_scalar(out=y0f[:, :nt, :], in0=py,
                                            scalar1=MAGIC, scalar2=MAGIC,
                                            op0=OP.add, op1=OP.subtract)
                    xb = x0f
                    yb = y0f
                    nc.vector.tensor_scalar_max(out=xb[:, :nt, :],
                                                in0=xb[:, :nt, :], scalar1=0.0)
                    nc.vector.tensor_tensor(out=xb[:, :nt, :], in0=xb[:, :nt, :],
                                            in1=bc(lwm1_c), op=OP.min)
                    nc.vector.tensor_scalar_max(out=yb[:, :nt, :],
                                                in0=yb[:, :nt, :], scalar1=0.0)
                    nc.vector.tensor_tensor(out=yb[:, :nt, :], in0=yb[:, :nt, :],
                                            in1=bc(lhm1_c), op=OP.min)

                    # slot weights: wq[axis, pm] = relu(1 - |d +/- 0.5|)
                    d2 = wpool.tile([128, CHUNK, 2, 16], F32, tag=f"d2{h}")
                    nc.vector.tensor_tensor(out=d2[:, :nt, 0, :], in0=px,
                                            in1=xb[:, :nt, :], op=OP.subtract)
                    nc.vector.tensor_tensor(out=d2[:, :nt, 1, :], in0=py,
                                            in1=yb[:, :nt, :], op=OP.subtract)
                    wq = wpool.tile([128, CHUNK, 2, 2, 16], F32, tag=f"wq{h}")
                    nc.scalar.activation(out=wq[:, :nt, :, 0, :],
                                         in_=d2[:, :nt, :, :],
                                         func=ACT.Abs, bias=bp05[:])
                    nc.scalar.activation(out=wq[:, :nt, :, 1, :],
                                         in_=d2[:, :nt, :, :],
                                         func=ACT.Abs, bias=bm05[:])
                    nc.scalar.activation(out=wq[:, :nt, :, :, :],
                                         in_=wq[:, :nt, :, :, :],
                                         func=ACT.Relu, scale=-1.0, bias=1.0)

                    mx = wpool.tile([128, CHUNK, 16], F32, tag=f"mx{h}")
                    my = wpool.tile([128, CHUNK, 16], F32, tag=f"my{h}")
                    nc.vector.tensor_tensor(out=mx[:, :nt, :], in0=px,
                                            in1=bc(lwm15_c), op=OP.is_lt)
                    nc.vector.tensor_tensor(out=my[:, :nt, :], in0=py,
                                            in1=bc(lhm15_c), op=OP.is_lt)
                    nc.vector.tensor_tensor(out=wq[:, :nt, 0, 1, :],
                                            in0=wq[:, :nt, 0, 1, :],
                                            in1=mx[:, :nt, :], op=OP.mult)
                    nc.vector.tensor_tensor(out=wq[:, :nt, 1, 1, :],
                                            in0=wq[:, :nt, 1, 1, :],
                                            in1=my[:, :nt, :], op=OP.mult)
                    nc.vector.tensor_tensor(out=wq[:, :nt, 1, 0, :],
                                            in0=wq[:, :nt, 1, 0, :],
                                            in1=e[:, :nt, :], op=OP.mult)
                    nc.vector.tensor_tensor(out=wq[:, :nt, 1, 1, :],
                                            in0=wq[:, :nt, 1, 1, :],
                                            in1=e[:, :nt, :], op=OP.mult)

                    # w4[q, (lp, slot)]  slot = dy*2+dx
                    w4h = wpool.tile([128, CHUNK, 16, 4], F32, tag=f"w4{h}")
                    for s, (ydx, xdx) in enumerate(
                            ((0, 0), (0, 1), (1, 0), (1, 1))):
                        nc.vector.tensor_tensor(out=w4h[:, :nt, :, s],
                                                in0=wq[:, :nt, 1, ydx, :],
                                                in1=wq[:, :nt, 0, xdx, :],
                                                op=OP.mult)
                    w4hc[h] = w4h

                    # idx f32 = start + yb*lw + xb
                    nc.vector.tensor_tensor(out=yb[:, :nt, :], in0=yb[:, :nt, :],
                                            in1=bc(lw_c), op=OP.mult)
                    nc.vector.tensor_tensor(out=xb[:, :nt, :], in0=xb[:, :nt, :],
                                            in1=yb[:, :nt, :], op=OP.add)
                    nc.vector.tensor_tensor(out=xb[:, :nt, :], in0=xb[:, :nt, :],
                                            in1=bc(start_c), op=OP.add)
                    # idx wrap per tile: sample order i = lp*128 + q, wrap
                    # slot [i%16, i//16] = [q%16, lp*8 + q//16].  R[q, lp, c]
                    # = idx[q, lp] * (q//16 == c); psw[m, (lp, c)] =
                    # sum_q (q%16 == m%16) R[q, lp, c] = idx[c*16+m%16, lp]
                    # -> 8 replicas across partitions (sim reads 0-15, HW
                    # queue k reads 32k..32k+32).
                    for i in range(nt):
                        rw = mpool.tile([128, 16, 8], F32, tag="rw")
                        nc.vector.tensor_tensor(
                            out=rw[:],
                            in0=xb[:, i, :].unsqueeze(2).to_broadcast(
                                [128, 16, 8]),
                            in1=maskc_c.unsqueeze(1).to_broadcast([128, 16, 8]),
                            op=OP.mult)
                        psw = psT.tile([128, 128], F32, tag="ps_t")
                        nc.tensor.matmul(
                            out=psw[:], lhsT=s128_c,
                            rhs=rw[:].rearrange("p l c -> p (l c)"),
                            start=True, stop=True)
                        i16 = ipool.tile([128, 128], I16, tag="i16")
                        nc.vector.tensor_copy(out=i16[:], in_=psw[:])
                        i16s[(t0, i, h)] = i16
                w4hs[t0] = w4hc

            def emit_p3(t0):
                nt = min(CHUNK, NT - t0)
                w4hc = w4hs.pop(t0)
                msda = wpool.tile([128, CHUNK, 64], F32, tag="msda")
                for i in range(nt):
                    t = t0 + i
                    for h in range(2):
                        i16 = i16s.pop((t0, i, h))
                        # g[q, lp, s, ch] (sample order i = lp*128 + q)
                        g = gpool.tile([128, 16, 4, 32], FP16, tag="g")
                        nc.gpsimd.dma_gather(
                            out_ap=g[:].rearrange("p a b c -> p a (b c)"),
                            in_ap=tabv[h], idxs_ap=i16[:],
                            num_idxs=2048, num_idxs_reg=ni_reg,
                            elem_size=128, single_packet=False,
                            queue_num=(t * 2 + h) % 4)
                        p16 = gpool.tile([128, 16, 4, 32], FP16, tag="p16")
                        nc.vector.tensor_tensor(
                            out=p16[:], in0=g[:],
                            in1=w4hc[h][:, i, :, :].unsqueeze(3).to_broadcast(
                                [128, 16, 4, 32]),
                            op=OP.mult)
                        # reduce slots (inner, stride 32) then lp (stride 32
                        # els in ssum) -> msda[q, 32ch]
                        ssum = gpool.tile([128, 16, 32], F32, tag="ssum")
                        nc.vector.tensor_reduce(
                            out=ssum[:],
                            in_=p16[:].rearrange("p l s c -> p l c s"),
                            axis=AX.X, op=OP.add)
                        nc.vector.tensor_reduce(
                            out=msda[:, i, h * 32:(h + 1) * 32],
                            in_=ssum[:].rearrange("p l c -> p c l"),
                            axis=AX.X, op=OP.add)
                return msda

            def emit_p4(t0, msda):
                nt = min(CHUNK, NT - t0)
                osb = wpool.tile([128, CHUNK, 256], F32, tag="osb")
                for i in range(nt):
                    psb = psB.tile([64, 128], F32, tag="ps_b")
                    nc.tensor.transpose(out=psb[:], in_=msda[:, i, :],
                                        identity=ident[:])
                    mT = mpool.tile([64, 128], F32, tag="mT")
                    nc.scalar.copy(out=mT[:], in_=psb[:])
                    pso = psB.tile([128, 256], F32, tag="ps_b")
                    nc.tensor.matmul(out=pso[:], lhsT=mT[:], rhs=wout_sb[:],
                                     start=True, stop=True)
                    nc.scalar.copy(out=osb[:, i, :], in_=pso[:])
                nc.scalar.dma_start(
                    out=outp[t0 * 128:(t0 + nt) * 128, :].rearrange(
                        "(t q) c -> q t c", q=128),
                    in_=osb[:, :nt, :])

            emit_p2(0)
            # ---- P1: value projection -> vtab ----
            def emit_p1b(lv):
                lh, lw = SPATIAL[lv]
                s0, hw = STARTS[lv], HWS[lv]
                for h in range(2):
                    for s, sh in enumerate((0, 1, lw, lw + 1)):
                        nc.sync.dma_start(
                            out=tab[h][s0:s0 + hw, s * 32:(s + 1) * 32],
                            in_=vtab[h][s0 + sh:s0 + sh + hw, :])

            t0 = 0
            while t0 < NT:
                nt = min(CHUNK, NT - t0)
                vas = []
                for i, (o, kk) in enumerate(KV_CH):
                    va = spool.tile([128, CHUNK * 128], BF16, tag="vld",
                                    name=f"vab{i}")
                    nc.sync.dma_start(
                        out=va[:kk, :nt * 128],
                        in_=vaug[o:o + kk, t0 * 128:(t0 + nt) * 128])
                    vas.append(va)
                vsb = mpool.tile([128, CHUNK, 64], FP16, tag="vsb")
                for i in range(nt):
                    psv = psA.tile([128, 64], F32, tag="ps_a")
                    for j, (o, kk) in enumerate(KV_CH):
                        nc.tensor.matmul(
                            out=psv[:], lhsT=vas[j][:kk, i * 128:(i + 1) * 128],
                            rhs=wv_sb[j][:],
                            start=(j == 0), stop=(j == len(KV_CH) - 1))
                    nc.scalar.copy(out=vsb[:, i, :], in_=psv[:])
                for h in range(2):
                    nc.sync.dma_start(
                        out=vtab[h][t0 * 128:(t0 + nt) * 128, :].rearrange(
                            "(t q) c -> q t c", q=128),
                        in_=vsb[:, :nt, h * 32:(h + 1) * 32])
                t0 += nt
                # interleave patch-table builds: emit each level's slot DMAs
                # as soon as the vtab rows it reads are stored
                if t0 == 72:
                    emit_p1b(0)
                elif t0 == 88:
                    emit_p1b(1)
            emit_p1b(2)
            emit_p1b(3)

            chunk_starts = list(range(0, NT, CHUNK))
            for ci, t0 in enumerate(chunk_starts):
                if ci + 1 < len(chunk_starts):
                    emit_p2(chunk_starts[ci + 1])
                msda = emit_p3(t0)
                emit_p4(t0, msda)
    nc.compile()
    return nc


def host_prep(inputs):
    q = np.asarray(inputs["query"], np.float32)
    ref = np.asarray(inputs["reference_points"], np.float32)
    val = np.asarray(inputs["value"], np.float32)
    W_off = np.asarray(inputs["W_off"], np.float32)
    b_off = np.asarray(inputs["b_off"], np.float32)
    W_attn = np.asarray(inputs["W_attn"], np.float32)
    b_attn = np.asarray(inputs["b_attn"], np.float32)
    W_val = np.asarray(inputs["W_val"], np.float32)
    b_val = np.asarray(inputs["b_val"], np.float32)
    W_out = np.asarray(inputs["W_out"], np.float32)

    lh = np.array([s[0] for s in SPATIAL], np.float32)
    lw = np.array([s[1] for s in SPATIAL], np.float32)

    qaug = np.zeros((B, KA, NQP), np.float32)
    for b in range(B):
        qaug[b, :DM, :NQ] = q[b].T
        rx = ref[b, :, :, 0] * lw[None, :] - 1.0   # px' = px - 0.5
        ry = ref[b, :, :, 1] * lh[None, :] - 1.0
        qaug[b, DM:DM + 16, :NQ] = np.repeat(rx, P, axis=1).T
        qaug[b, DM + 16:DM + 32, :NQ] = np.repeat(ry, P, axis=1).T
        qaug[b, DM + 32, :] = 1.0

    vaug = np.zeros((B, KV, NQP), np.float32)
    for b in range(B):
        vaug[b, :DM, :NV] = val[b].T
        vaug[b, DM, :] = 1.0

    W_off_r = W_off.reshape(DM, H, L, P, 2)
    b_off_r = b_off.reshape(H, L, P, 2)
    W_attn_r = W_attn.reshape(DM, H, L, P)
    b_attn_r = b_attn.reshape(H, L, P)

    waug_all = np.zeros((H, KA, 48), np.float32)
    for h in range(H):
        waug_all[h, :DM, 0:16] = W_off_r[:, h, :, :, 0].reshape(DM, 16)
        waug_all[h, :DM, 16:32] = W_off_r[:, h, :, :, 1].reshape(DM, 16)
        waug_all[h, :DM, 32:48] = W_attn_r[:, h].reshape(DM, 16)
        waug_all[h, DM + 32, 0:16] = b_off_r[h, :, :, 0].reshape(16)
        waug_all[h, DM + 32, 16:32] = b_off_r[h, :, :, 1].reshape(16)
        waug_all[h, DM + 32, 32:48] = b_attn_r[h].reshape(16)
        for j in range(16):
            waug_all[h, DM + j, j] = 1.0
            waug_all[h, DM + 16 + j, 16 + j] = 1.0

    W_val_r = W_val.reshape(DM, H, HD)
    b_val_r = b_val.reshape(H, HD)

    consts = np.zeros((128, 6, 16), np.float32)
    consts[:, 0, :] = np.repeat(lw, P)[None, :]
    consts[:, 1, :] = np.repeat(lw - 1.0, P)[None, :]
    consts[:, 2, :] = np.repeat(lh - 1.0, P)[None, :]
    consts[:, 3, :] = np.repeat(np.array(STARTS, np.float32), P)[None, :]
    consts[:, 4, :] = np.repeat(lw - 1.5, P)[None, :]
    consts[:, 5, :] = np.repeat(lh - 1.5, P)[None, :]

    wrapc = np.zeros((128, 136), np.float32)
    for q in range(128):
        for m in range(128):
            if q % 16 == m % 16:
                wrapc[q, m] = 1.0
        wrapc[q, 128 + q // 16] = 1.0

    in_maps = []
    for c in range(N_CORES):
        b = c // 4
        h0 = 2 * (c % 4)
        waug = np.concatenate([waug_all[h0], waug_all[h0 + 1]], axis=1)
        wv = np.zeros((KV, 64), np.float32)
        wv[:DM, 0:32] = W_val_r[:, h0, :]
        wv[:DM, 32:64] = W_val_r[:, h0 + 1, :]
        wv[DM, 0:32] = b_val_r[h0]
        wv[DM, 32:64] = b_val_r[h0 + 1]
        wo = np.ascontiguousarray(
            W_out.reshape(H, HD, DM)[h0:h0 + 2].reshape(64, DM))
        in_maps.append({
            "qaug": np.ascontiguousarray(qaug[b]),
            "vaug": np.ascontiguousarray(vaug[b]).astype(ml_dtypes.bfloat16),
            "waug": np.ascontiguousarray(waug),
            "wvaug": wv.astype(ml_dtypes.bfloat16),
            "wout": wo,
            "consts": consts,
            "wrapc": wrapc,
        })
    return in_maps


_NC_CACHE = None


def kernel(**inputs) -> np.ndarray:
    global _NC_CACHE
    in_maps = host_prep(inputs)
    if _NC_CACHE is None:
        _NC_CACHE = build_module()
    nc = _NC_CACHE
    res = run_bass_kernel_spmd(nc, in_maps, core_ids=list(range(N_CORES)))
    b_out = np.asarray(inputs["b_out"], np.float32)
    out = np.zeros((B, NQ, DM), np.float32)
    for c in range(N_CORES):
        out[c // 4] += res.results[c]["outp"][:NQ, :]
    out += b_out[None, None, :]
    return out


if __name__ == "__main__":
    import reference

    inputs = {k: np.asarray(v) for k, v in reference.setup_inputs().items()}
    got = kernel(**inputs)
    exp = np.asarray(reference.reference(**inputs))
    err = np.abs(got - exp)
    rel = np.linalg.norm(got - exp) / np.linalg.norm(exp)
    print("abs max err:", err.max(), "rel:", rel)



# revision 25
# speedup vs baseline: 1.4009x; 1.4009x over previous
"""Multi-Scale Deformable Attention (DigitDETR encoder layer) on 8 TRN2 cores.

Sharding: 16 (batch, head) pairs over 8 cores -> each core handles one batch
and two consecutive heads (data-parallel over B, tensor-parallel over H).
Each core computes a partial output  msda(b, h0..h1) @ W_out[h-rows]; the host
sums the 4 partials per batch and adds b_out during unsharding.

Per-core device pipeline (Tile framework):
  P1  value projection (PE) -> v[h] in DRAM -> column-major x-duplicated
      "patch table" per head: row r = pixel (x, y), content = [v(y,x), v(y,x+1)]
      (64 f32).  Rows r, r+1 are y-neighbors, so one 512B read at row r
      yields the full 2x2 bilinear patch (elem_size=128, elem_step=64).
  P2  fused projection matmul (query_T | ref*size-1.0 | ones) @ W_aug gives
      px' (= px-0.5), py', attn logits; softmax; floor via +/-2^23 magic;
      clip; slot weights relu(1-|d±.5|) with border masks; idx f32.
  P3  gather via dma_gather with sample order i = lp*128+q so G lands
      q-partitioned [q, lp, 4, 32]; idx wrap [q%16, lp*8+q//16] built by a
      masked-replicate matmul (R = idx*maskC; psw = S128 @ R -> 8 replicas
      for sim partitions 0-15 + all 4 SWDGE queue core pairs); gathers
      round-robin queues 0-3 so 4 Q7 pairs generate descriptors in
      parallel; DVE product with w4 read straight from P2 (no DRAM
      bounce); 2 DVE reduces (slots, then lp) -> msda[q, 64] in SBUF.
  P4  PE-transpose msda, out-projection matmul -> rows.
"""

import ml_dtypes
import numpy as np

import concourse.bass as bass
import concourse.bacc as bacc
import concourse.mybir as mybir
import concourse.tile as tile
from concourse.bass_utils import run_bass_kernel_spmd
from concourse.masks import make_identity

F32 = mybir.dt.float32
FP16 = mybir.dt.float16
BF16 = mybir.dt.bfloat16
I16 = mybir.dt.int16
AX = mybir.AxisListType
OP = mybir.AluOpType
ACT = mybir.ActivationFunctionType

# ---- static problem config ----
SPATIAL = ((76, 114), (38, 57), (19, 29), (10, 15))  # (lh, lw)
HWS = [h * w for h, w in SPATIAL]
STARTS = [0, 8664, 10830, 11381]
NV = 11531
B, H, L, P, DM, HD = 2, 8, 4, 4, 256, 32
NQ = NV
NT = 91
NQP = NT * 128          # 11648
KA = DM + 2 * L * P + 1  # 289
KV = DM + 1              # 257
MAGIC = 12582912.0       # 1.5 * 2^23
CHUNK = 8

N_CORES = 8


def _kchunks(k):
    out, o = [], 0
    while o < k:
        kk = min(128, k - o)
        out.append((o, kk))
        o += kk
    return out


KA_CH = _kchunks(KA)
KV_CH = _kchunks(KV)


def build_module(reps=1):
    nc = bacc.Bacc("TRN2", target_bir_lowering=False, debug=False,
                   enable_asserts=False, num_devices=N_CORES,
                   num_swdge_queues=4, dynamic_dma_scratch_size=40960)

    qaug = nc.dram_tensor("qaug", [KA, NQP], F32, kind="ExternalInput").ap()
    vaug = nc.dram_tensor("vaug", [KV, NQP], BF16, kind="ExternalInput").ap()
    waug = nc.dram_tensor("waug", [KA, 96], F32, kind="ExternalInput").ap()
    wvaug = nc.dram_tensor("wvaug", [KV, 64], BF16, kind="ExternalInput").ap()
    wout = nc.dram_tensor("wout", [64, 256], F32, kind="ExternalInput").ap()
    # rows: 0=lw, 1=lw-1, 2=lh-1, 3=start, 4=lw-1.5, 5=lh-1.5
    consts = nc.dram_tensor("consts", [128, 6, 16], F32, kind="ExternalInput").ap()
    # wrapc[:, :128] = S128[q, m] = (q%16 == m%16); [:, 128:136] = maskC[q, c]
    # = (q//16 == c)
    wrapc = nc.dram_tensor("wrapc", [128, 136], F32, kind="ExternalInput").ap()
    outp = nc.dram_tensor("outp", [NQP, 256], F32, kind="ExternalOutput").ap()

    with tile.TileContext(nc) as tc:
        with (
            tc.tile_pool(name="dram", bufs=1, space="DRAM") as dpool,
            tc.tile_pool(name="const", bufs=1) as cpool,
            tc.tile_pool(name="stat", bufs=6) as spool,
            tc.tile_pool(name="work", bufs=2) as wpool,
            tc.tile_pool(name="gbuf", bufs=10) as gpool,
            tc.tile_pool(name="pbuf", bufs=4) as ppool,
            tc.tile_pool(name="small", bufs=3) as mpool,
            tc.tile_pool(name="idx", bufs=18) as ipool,
            tc.tile_pool(name="psA", bufs=2, space="PSUM") as psA,
            tc.tile_pool(name="psT", bufs=2, space="PSUM") as psT,
            tc.tile_pool(name="psB", bufs=2, space="PSUM") as psB,
        ):
            # ---- resident constants ----
            ident = cpool.tile([128, 128], F32, tag="ident")
            make_identity(nc, ident[:])
            ident16 = cpool.tile([128, 128], FP16, tag="ident16")
            make_identity(nc, ident16[:])
            const_sb = cpool.tile([128, 6, 16], F32, tag="consts")
            nc.sync.dma_start(out=const_sb[:], in_=consts)
            lw_c = const_sb[:, 0, :]
            lwm1_c = const_sb[:, 1, :]
            lhm1_c = const_sb[:, 2, :]
            start_c = const_sb[:, 3, :]
            lwm15_c = const_sb[:, 4, :]
            lhm15_c = const_sb[:, 5, :]

            wrap_sb = cpool.tile([128, 136], F32, tag="wrapc")
            nc.sync.dma_start(out=wrap_sb[:], in_=wrapc)
            s128_c = wrap_sb[:, 0:128]
            maskc_c = wrap_sb[:, 128:136]

            wa_sb = []
            for i, (o, kk) in enumerate(KA_CH):
                t = cpool.tile([kk, 96], F32, tag=f"wa{i}", name=f"wa{i}")
                nc.sync.dma_start(out=t[:], in_=waug[o:o + kk, :])
                wa_sb.append(t)
            wv_sb = []
            for i, (o, kk) in enumerate(KV_CH):
                t = cpool.tile([kk, 64], BF16, tag=f"wv{i}", name=f"wv{i}")
                nc.sync.dma_start(out=t[:], in_=wvaug[o:o + kk, :])
                wv_sb.append(t)
            wout_sb = cpool.tile([64, 256], F32, tag="wout")
            nc.sync.dma_start(out=wout_sb[:], in_=wout)

            bp05 = cpool.tile([128, 1], F32, tag="bp05")
            nc.vector.memset(bp05[:], 0.5)
            bm05 = cpool.tile([128, 1], F32, tag="bm05")
            nc.vector.memset(bm05[:], -0.5)

            ni_reg = nc.gpsimd.to_reg(2048)

            # ---- DRAM scratch ----
            vtab = [dpool.tile([NQP, 32], FP16, tag=f"vtab{h}", name=f"vtab{h}")
                    for h in range(2)]
            tab = [dpool.tile([NQP, 128], FP16, tag=f"tab{h}", name=f"tab{h}")
                   for h in range(2)]
            tabv = [tab[h][:] for h in range(2)]

            # ---- pipelined P2/P3/P4: emission order = engine-queue order,
            # so chunk c+1's projections/indices are emitted before chunk c's
            # gather consumers to keep every engine fed while Pool gathers. ----
            i16s = {}    # (chunk_t0, i, h) -> i16 tile
            w4hs = {}    # chunk_t0 -> [w4h_h0, w4h_h1]

            def emit_p2(t0):
                nt = min(CHUNK, NT - t0)
                proj = wpool.tile([128, CHUNK, 96], F32, tag="proj")
                qas = []
                for j, (o, kk) in enumerate(KA_CH):
                    qa = spool.tile([128, CHUNK * 128], F32, tag="ld",
                                    name=f"qab{j}")
                    nc.scalar.dma_start(
                        out=qa[:kk, :nt * 128],
                        in_=qaug[o:o + kk, t0 * 128:(t0 + nt) * 128])
                    qas.append(qa)
                for i in range(nt):
                    psp = psA.tile([128, 96], F32, tag="ps_a")
                    for j, (o, kk) in enumerate(KA_CH):
                        nc.tensor.matmul(
                            out=psp[:], lhsT=qas[j][:kk, i * 128:(i + 1) * 128],
                            rhs=wa_sb[j][:],
                            start=(j == 0), stop=(j == len(KA_CH) - 1))
                    nc.scalar.copy(out=proj[:, i, :], in_=psp[:])

                w4hc = [None, None]
                for h in range(2):
                    c0 = h * 48
                    px = proj[:, :nt, c0 + 0:c0 + 16]
                    py = proj[:, :nt, c0 + 16:c0 + 32]
                    att = proj[:, :nt, c0 + 32:c0 + 48]

                    def bc(c16):
                        return c16.unsqueeze(1).to_broadcast([128, nt, 16])

                    # softmax over 16 (l,p)
                    e = wpool.tile([128, CHUNK, 16], F32, tag=f"e{h}")
                    nc.scalar.activation(out=e[:, :nt, :], in_=att, func=ACT.Exp)
                    ssum = mpool.tile([128, CHUNK], F32, tag=f"ss{h}")
                    nc.vector.tensor_reduce(out=ssum[:, :nt], in_=e[:, :nt, :],
                                            axis=AX.X, op=OP.add)
                    rinv = mpool.tile([128, CHUNK], F32, tag=f"ri{h}")
                    nc.vector.reciprocal(out=rinv[:, :nt], in_=ssum[:, :nt])
                    nc.vector.tensor_tensor(
                        out=e[:, :nt, :], in0=e[:, :nt, :],
                        in1=rinv[:, :nt].unsqueeze(2).to_broadcast([128, nt, 16]),
                        op=OP.mult)

                    # floor/clip
                    x0f = wpool.tile([128, CHUNK, 16], F32, tag=f"x0{h}")
                    y0f = wpool.tile([128, CHUNK, 16], F32, tag=f"y0{h}")
                    nc.vector.tensor_scalar(out=x0f[:, :nt, :], in0=px,
                                            scalar1=MAGIC, scalar2=MAGIC,
                                            op0=OP.add, op1=OP.subtract)
                    nc.vector.tensor_scalar(out=y0f[:, :nt, :], in0=py,
                                            scalar1=MAGIC, scalar2=MAGIC,
                                            op0=OP.add, op1=OP.subtract)
                    xb = x0f
                    yb = y0f
                    nc.vector.tensor_scalar_max(out=xb[:, :nt, :],
                                                in0=xb[:, :nt, :], scalar1=0.0)
                    nc.vector.tensor_tensor(out=xb[:, :nt, :], in0=xb[:, :nt, :],
                                            in1=bc(lwm1_c), op=OP.min)
                    nc.vector.tensor_scalar_max(out=yb[:, :nt, :],
                                                in0=yb[:, :nt, :], scalar1=0.0)
                    nc.vector.tensor_tensor(out=yb[:, :nt, :], in0=yb[:, :nt, :],
                                            in1=bc(lhm1_c), op=OP.min)

                    # slot weights: wq[axis, pm] = relu(1 - |d +/- 0.5|)
                    d2 = wpool.tile([128, CHUNK, 2, 16], F32, tag=f"d2{h}")
                    nc.vector.tensor_tensor(out=d2[:, :nt, 0, :], in0=px,
                                            in1=xb[:, :nt, :], op=OP.subtract)
                    nc.vector.tensor_tensor(out=d2[:, :nt, 1, :], in0=py,
                                            in1=yb[:, :nt, :], op=OP.subtract)
                    wq = wpool.tile([128, CHUNK, 2, 2, 16], F32, tag=f"wq{h}")
                    nc.scalar.activation(out=wq[:, :nt, :, 0, :],
                                         in_=d2[:, :nt, :, :],
                                         func=ACT.Abs, bias=bp05[:])
                    nc.scalar.activation(out=wq[:, :nt, :, 1, :],
                                         in_=d2[:, :nt, :, :],
                                         func=ACT.Abs, bias=bm05[:])
                    nc.scalar.activation(out=wq[:, :nt, :, :, :],
                                         in_=wq[:, :nt, :, :, :],
                                         func=ACT.Relu, scale=-1.0, bias=1.0)

                    mx = wpool.tile([128, CHUNK, 16], F32, tag=f"mx{h}")
                    my = wpool.tile([128, CHUNK, 16], F32, tag=f"my{h}")
                    nc.vector.tensor_tensor(out=mx[:, :nt, :], in0=px,
                                            in1=bc(lwm15_c), op=OP.is_lt)
                    nc.vector.tensor_tensor(out=my[:, :nt, :], in0=py,
                                            in1=bc(lhm15_c), op=OP.is_lt)
                    nc.vector.tensor_tensor(out=wq[:, :nt, 0, 1, :],
                                            in0=wq[:, :nt, 0, 1, :],
                                            in1=mx[:, :nt, :], op=OP.mult)
                    nc.vector.tensor_tensor(out=wq[:, :nt, 1, 1, :],
                                            in0=wq[:, :nt, 1, 1, :],
                                            in1=my[:, :nt, :], op=OP.mult)
                    nc.vector.tensor_tensor(out=wq[:, :nt, 1, 0, :],
                                            in0=wq[:, :nt, 1, 0, :],
                                            in1=e[:, :nt, :], op=OP.mult)
                    nc.vector.tensor_tensor(out=wq[:, :nt, 1, 1, :],
                                            in0=wq[:, :nt, 1, 1, :],
                                            in1=e[:, :nt, :], op=OP.mult)

                    # w4[q, (lp, slot)]  slot = dy*2+dx
                    w4h = wpool.tile([128, CHUNK, 16, 4], F32, tag=f"w4{h}")
                    for s, (ydx, xdx) in enumerate(
                            ((0, 0), (0, 1), (1, 0), (1, 1))):
                        nc.vector.tensor_tensor(out=w4h[:, :nt, :, s],
                                                in0=wq[:, :nt, 1, ydx, :],
                                                in1=wq[:, :nt, 0, xdx, :],
                                                op=OP.mult)
                    w4hc[h] = w4h

                    # idx f32 = start + yb*lw + xb
                    nc.vector.tensor_tensor(out=yb[:, :nt, :], in0=yb[:, :nt, :],
                                            in1=bc(lw_c), op=OP.mult)
                    nc.vector.tensor_tensor(out=xb[:, :nt, :], in0=xb[:, :nt, :],
                                            in1=yb[:, :nt, :], op=OP.add)
                    nc.vector.tensor_tensor(out=xb[:, :nt, :], in0=xb[:, :nt, :],
                                            in1=bc(start_c), op=OP.add)
                    # idx wrap per tile: sample order i = lp*128 + q, wrap
                    # slot [i%16, i//16] = [q%16, lp*8 + q//16].  R[q, lp, c]
                    # = idx[q, lp] * (q//16 == c); psw[m, (lp, c)] =
                    # sum_q (q%16 == m%16) R[q, lp, c] = idx[c*16+m%16, lp]
                    # -> 8 replicas across partitions (sim reads 0-15, HW
                    # queue k reads 32k..32k+32).
                    for i in range(nt):
                        rw = mpool.tile([128, 16, 8], F32, tag="rw")
                        nc.vector.tensor_tensor(
                            out=rw[:],
                            in0=xb[:, i, :].unsqueeze(2).to_broadcast(
                                [128, 16, 8]),
                            in1=maskc_c.unsqueeze(1).to_broadcast([128, 16, 8]),
                            op=OP.mult)
                        psw = psT.tile([128, 128], F32, tag="ps_t")
                        nc.tensor.matmul(
                            out=psw[:], lhsT=s128_c,
                            rhs=rw[:].rearrange("p l c -> p (l c)"),
                            start=True, stop=True)
                        i16 = ipool.tile([128, 128], I16, tag="i16")
                        nc.vector.tensor_copy(out=i16[:], in_=psw[:])
                        i16s[(t0, i, h)] = i16
                w4hs[t0] = w4hc

            def emit_p3(t0):
                nt = min(CHUNK, NT - t0)
                w4hc = w4hs.pop(t0)
                msda = wpool.tile([128, CHUNK, 64], F32, tag="msda")
                for i in range(nt):
                    t = t0 + i
                    for h in range(2):
                        i16 = i16s.pop((t0, i, h))
                        # g[q, lp, s, ch] (sample order i = lp*128 + q)
                        g = gpool.tile([128, 16, 4, 32], FP16, tag="g")
                        nc.gpsimd.dma_gather(
                            out_ap=g[:].rearrange("p a b c -> p a (b c)"),
                            in_ap=tabv[h], idxs_ap=i16[:],
                            num_idxs=2048, num_idxs_reg=ni_reg,
                            elem_size=128, single_packet=False,
                            queue_num=(t * 2 + h) % 4)
                        p16 = ppool.tile([128, 16, 4, 32], FP16, tag="p16")
                        nc.vector.tensor_tensor(
                            out=p16[:], in0=g[:],
                            in1=w4hc[h][:, i, :, :].unsqueeze(3).to_broadcast(
                                [128, 16, 4, 32]),
                            op=OP.mult)
                        # slot sum: 3 contiguous-run adds -> ssum[q, lp, ch]
                        ssum = ppool.tile([128, 16, 32], FP16, tag="ssum")
                        nc.vector.tensor_tensor(
                            out=ssum[:], in0=p16[:, :, 0, :],
                            in1=p16[:, :, 1, :], op=OP.add)
                        nc.vector.tensor_tensor(
                            out=ssum[:], in0=ssum[:],
                            in1=p16[:, :, 2, :], op=OP.add)
                        nc.vector.tensor_tensor(
                            out=ssum[:], in0=ssum[:],
                            in1=p16[:, :, 3, :], op=OP.add)
                        # lp sum: contiguous halving tree (fp16) -> fp32 msda
                        nc.vector.tensor_tensor(
                            out=ssum[:, :8, :], in0=ssum[:, :8, :],
                            in1=ssum[:, 8:, :], op=OP.add)
                        nc.vector.tensor_tensor(
                            out=ssum[:, :4, :], in0=ssum[:, :4, :],
                            in1=ssum[:, 4:8, :], op=OP.add)
                        nc.vector.tensor_tensor(
                            out=ssum[:, :2, :], in0=ssum[:, :2, :],
                            in1=ssum[:, 2:4, :], op=OP.add)
                        nc.vector.tensor_tensor(
                            out=msda[:, i, h * 32:(h + 1) * 32],
                            in0=ssum[:, 0, :], in1=ssum[:, 1, :], op=OP.add)
                return msda

            def emit_p4(t0, msda):
                nt = min(CHUNK, NT - t0)
                osb = wpool.tile([128, CHUNK, 256], F32, tag="osb")
                for i in range(nt):
                    psb = psB.tile([64, 128], F32, tag="ps_b")
                    nc.tensor.transpose(out=psb[:], in_=msda[:, i, :],
                                        identity=ident[:])
                    mT = mpool.tile([64, 128], F32, tag="mT")
                    nc.scalar.copy(out=mT[:], in_=psb[:])
                    pso = psB.tile([128, 256], F32, tag="ps_o")
                    nc.tensor.matmul(out=pso[:], lhsT=mT[:], rhs=wout_sb[:],
                                     start=True, stop=True)
                    nc.scalar.copy(out=osb[:, i, :], in_=pso[:])
                nc.scalar.dma_start(
                    out=outp[t0 * 128:(t0 + nt) * 128, :].rearrange(
                        "(t q) c -> q t c", q=128),
                    in_=osb[:, :nt, :])

            emit_p2(0)
            # ---- P1: value projection -> vtab ----
            def emit_p1b(lv):
                lh, lw = SPATIAL[lv]
                s0, hw = STARTS[lv], HWS[lv]
                for h in range(2):
                    for s, sh in enumerate((0, 1, lw, lw + 1)):
                        nc.sync.dma_start(
                            out=tab[h][s0:s0 + hw, s * 32:(s + 1) * 32],
                            in_=vtab[h][s0 + sh:s0 + sh + hw, :])

            t0 = 0
            while t0 < NT:
                nt = min(CHUNK, NT - t0)
                vas = []
                for i, (o, kk) in enumerate(KV_CH):
                    va = spool.tile([128, CHUNK * 128], BF16, tag="vld",
                                    name=f"vab{i}")
                    nc.sync.dma_start(
                        out=va[:kk, :nt * 128],
                        in_=vaug[o:o + kk, t0 * 128:(t0 + nt) * 128])
                    vas.append(va)
                vsb = mpool.tile([128, CHUNK, 64], FP16, tag="vsb")
                for i in range(nt):
                    psv = psA.tile([128, 64], F32, tag="ps_a")
                    for j, (o, kk) in enumerate(KV_CH):
                        nc.tensor.matmul(
                            out=psv[:], lhsT=vas[j][:kk, i * 128:(i + 1) * 128],
                            rhs=wv_sb[j][:],
                            start=(j == 0), stop=(j == len(KV_CH) - 1))
                    nc.scalar.copy(out=vsb[:, i, :], in_=psv[:])
                for h in range(2):
                    nc.sync.dma_start(
                        out=vtab[h][t0 * 128:(t0 + nt) * 128, :].rearrange(
                            "(t q) c -> q t c", q=128),
                        in_=vsb[:, :nt, h * 32:(h + 1) * 32])
                t0 += nt
                # interleave patch-table builds: emit each level's slot DMAs
                # as soon as the vtab rows it reads are stored
                if t0 == 72:
                    emit_p1b(0)
                elif t0 == 88:
                    emit_p1b(1)
            emit_p1b(2)
            emit_p1b(3)

            chunk_starts = list(range(0, NT, CHUNK))
            for ci, t0 in enumerate(chunk_starts):
                if ci + 1 < len(chunk_starts):
                    emit_p2(chunk_starts[ci + 1])
                msda = emit_p3(t0)
                emit_p4(t0, msda)
    nc.compile()
    return nc


def host_prep(inputs):
    q = np.asarray(inputs["query"], np.float32)
    ref = np.asarray(inputs["reference_points"], np.float32)
    val = np.asarray(inputs["value"], np.float32)
    W_off = np.asarray(inputs["W_off"], np.float32)
    b_off = np.asarray(inputs["b_off"], np.float32)
    W_attn = np.asarray(inputs["W_attn"], np.float32)
    b_attn = np.asarray(inputs["b_attn"], np.float32)
    W_val = np.asarray(inputs["W_val"], np.float32)
    b_val = np.asarray(inputs["b_val"], np.float32)
    W_out = np.asarray(inputs["W_out"], np.float32)

    lh = np.array([s[0] for s in SPATIAL], np.float32)
    lw = np.array([s[1] for s in SPATIAL], np.float32)

    qaug = np.zeros((B, KA, NQP), np.float32)
    for b in range(B):
        qaug[b, :DM, :NQ] = q[b].T
        rx = ref[b, :, :, 0] * lw[None, :] - 1.0   # px' = px - 0.5
        ry = ref[b, :, :, 1] * lh[None, :] - 1.0
        qaug[b, DM:DM + 16, :NQ] = np.repeat(rx, P, axis=1).T
        qaug[b, DM + 16:DM + 32, :NQ] = np.repeat(ry, P, axis=1).T
        qaug[b, DM + 32, :] = 1.0

    vaug = np.zeros((B, KV, NQP), np.float32)
    for b in range(B):
        vaug[b, :DM, :NV] = val[b].T
        vaug[b, DM, :] = 1.0

    W_off_r = W_off.reshape(DM, H, L, P, 2)
    b_off_r = b_off.reshape(H, L, P, 2)
    W_attn_r = W_attn.reshape(DM, H, L, P)
    b_attn_r = b_attn.reshape(H, L, P)

    waug_all = np.zeros((H, KA, 48), np.float32)
    for h in range(H):
        waug_all[h, :DM, 0:16] = W_off_r[:, h, :, :, 0].reshape(DM, 16)
        waug_all[h, :DM, 16:32] = W_off_r[:, h, :, :, 1].reshape(DM, 16)
        waug_all[h, :DM, 32:48] = W_attn_r[:, h].reshape(DM, 16)
        waug_all[h, DM + 32, 0:16] = b_off_r[h, :, :, 0].reshape(16)
        waug_all[h, DM + 32, 16:32] = b_off_r[h, :, :, 1].reshape(16)
        waug_all[h, DM + 32, 32:48] = b_attn_r[h].reshape(16)
        for j in range(16):
            waug_all[h, DM + j, j] = 1.0
            waug_all[h, DM + 16 + j, 16 + j] = 1.0

    W_val_r = W_val.reshape(DM, H, HD)
    b_val_r = b_val.reshape(H, HD)

    consts = np.zeros((128, 6, 16), np.float32)
    consts[:, 0, :] = np.repeat(lw, P)[None, :]
    consts[:, 1, :] = np.repeat(lw - 1.0, P)[None, :]
    consts[:, 2, :] = np.repeat(lh - 1.0, P)[None, :]
    consts[:, 3, :] = np.repeat(np.array(STARTS, np.float32), P)[None, :]
    consts[:, 4, :] = np.repeat(lw - 1.5, P)[None, :]
    consts[:, 5, :] = np.repeat(lh - 1.5, P)[None, :]

    wrapc = np.zeros((128, 136), np.float32)
    for q in range(128):
        for m in range(128):
            if q % 16 == m % 16:
                wrapc[q, m] = 1.0
        wrapc[q, 128 + q // 16] = 1.0

    in_maps = []
    for c in range(N_CORES):
        b = c // 4
        h0 = 2 * (c % 4)
        waug = np.concatenate([waug_all[h0], waug_all[h0 + 1]], axis=1)
        wv = np.zeros((KV, 64), np.float32)
        wv[:DM, 0:32] = W_val_r[:, h0, :]
        wv[:DM, 32:64] = W_val_r[:, h0 + 1, :]
        wv[DM, 0:32] = b_val_r[h0]
        wv[DM, 32:64] = b_val_r[h0 + 1]
        wo = np.ascontiguousarray(
            W_out.reshape(H, HD, DM)[h0:h0 + 2].reshape(64, DM))
        in_maps.append({
            "qaug": np.ascontiguousarray(qaug[b]),
            "vaug": np.ascontiguousarray(vaug[b]).astype(ml_dtypes.bfloat16),
            "waug": np.ascontiguousarray(waug),
            "wvaug": wv.astype(ml_dtypes.bfloat16),
            "wout": wo,
            "consts": consts,
            "wrapc": wrapc,
        })
    return in_maps


_NC_CACHE = None


def kernel(**inputs) -> np.ndarray:
    global _NC_CACHE
    in_maps = host_prep(inputs)
    if _NC_CACHE is None:
        _NC_CACHE = build_module()
    nc = _NC_CACHE
    res = run_bass_kernel_spmd(nc, in_maps, core_ids=list(range(N_CORES)))
    b_out = np.asarray(inputs["b_out"], np.float32)
    out = np.zeros((B, NQ, DM), np.float32)
    for c in range(N_CORES):
        out[c // 4] += res.results[c]["outp"][:NQ, :]
    out += b_out[None, None, :]
    return out


if __name__ == "__main__":
    import reference

    inputs = {k: np.asarray(v) for k, v in reference.setup_inputs().items()}
    got = kernel(**inputs)
    exp = np.asarray(reference.reference(**inputs))
    err = np.abs(got - exp)
    rel = np.linalg.norm(got - exp) / np.linalg.norm(exp)
    print("abs max err:", err.max(), "rel:", rel)



# revision 31
# speedup vs baseline: 1.5801x; 1.1279x over previous
"""Multi-Scale Deformable Attention (DigitDETR encoder layer) on 8 TRN2 cores.

Sharding: 16 (batch, head) pairs over 8 cores -> each core handles one batch
and two consecutive heads (data-parallel over B, tensor-parallel over H).
Each core computes a partial output  msda(b, h0..h1) @ W_out[h-rows]; the host
sums the 4 partials per batch and adds b_out during unsharding.

Per-core device pipeline (Tile framework):
  P1  value projection (PE) -> v[h] in DRAM -> column-major x-duplicated
      "patch table" per head: row r = pixel (x, y), content = [v(y,x), v(y,x+1)]
      (64 f32).  Rows r, r+1 are y-neighbors, so one 512B read at row r
      yields the full 2x2 bilinear patch (elem_size=128, elem_step=64).
  P2  fused projection matmul (query_T | ref*size-1.0 | ones) @ W_aug gives
      px' (= px-0.5), py', attn logits; softmax; floor via +/-2^23 magic;
      clip; slot weights relu(1-|d±.5|) with border masks; idx f32.
  P3  gather via dma_gather with sample order i = lp*128+q so G lands
      q-partitioned [q, lp, 4, 32]; idx wrap [q%16, lp*8+q//16] built by a
      masked-replicate matmul (R = idx*maskC; psw = S128 @ R -> 8 replicas
      for sim partitions 0-15 + all 4 SWDGE queue core pairs); gathers
      round-robin queues 0-3 so 4 Q7 pairs generate descriptors in
      parallel; DVE product with w4 read straight from P2 (no DRAM
      bounce); 2 DVE reduces (slots, then lp) -> msda[q, 64] in SBUF.
  P4  PE-transpose msda, out-projection matmul -> rows.
"""

import ml_dtypes
import numpy as np

import concourse.bass as bass
import concourse.bacc as bacc
import concourse.mybir as mybir
import concourse.tile as tile
from concourse.bass_utils import run_bass_kernel_spmd
from concourse.masks import make_identity

F32 = mybir.dt.float32
FP16 = mybir.dt.float16
BF16 = mybir.dt.bfloat16
I16 = mybir.dt.int16
AX = mybir.AxisListType
OP = mybir.AluOpType
ACT = mybir.ActivationFunctionType

# ---- static problem config ----
SPATIAL = ((76, 114), (38, 57), (19, 29), (10, 15))  # (lh, lw)
HWS = [h * w for h, w in SPATIAL]
STARTS = [0, 8664, 10830, 11381]
NV = 11531
B, H, L, P, DM, HD = 2, 8, 4, 4, 256, 32
NQ = NV
NT = 91
NQP = NT * 128          # 11648
KA = DM + 2 * L * P + 1  # 289
KV = DM + 1              # 257
MAGIC = 12582912.0       # 1.5 * 2^23
CHUNK = 8

N_CORES = 8


def _kchunks(k):
    out, o = [], 0
    while o < k:
        kk = min(128, k - o)
        out.append((o, kk))
        o += kk
    return out


KA_CH = _kchunks(KA)
KV_CH = _kchunks(KV)


def build_module(reps=1):
    nc = bacc.Bacc("TRN2", target_bir_lowering=False, debug=False,
                   enable_asserts=False, num_devices=N_CORES,
                   num_swdge_queues=4, dynamic_dma_scratch_size=40960)

    qaug = nc.dram_tensor("qaug", [KA, NQP], F32, kind="ExternalInput").ap()
    vaug = nc.dram_tensor("vaug", [KV, NQP], BF16, kind="ExternalInput").ap()
    waug = nc.dram_tensor("waug", [KA, 96], F32, kind="ExternalInput").ap()
    wvaug = nc.dram_tensor("wvaug", [KV, 64], BF16, kind="ExternalInput").ap()
    wout = nc.dram_tensor("wout", [64, 256], F32, kind="ExternalInput").ap()
    # rows: 0=lw, 1=lw-1, 2=lh-1, 3=start, 4=lw-1.5, 5=lh-1.5
    consts = nc.dram_tensor("consts", [128, 6, 16], F32, kind="ExternalInput").ap()
    # wrapc[:, :128] = S128[q, m] = (q%16 == m%16); [:, 128:136] = maskC[q, c]
    # = (q//16 == c)
    wrapc = nc.dram_tensor("wrapc", [128, 136], F32, kind="ExternalInput").ap()
    outp = nc.dram_tensor("outp", [NQP, 256], F32, kind="ExternalOutput").ap()

    with tile.TileContext(nc) as tc:
        with (
            tc.tile_pool(name="dram", bufs=1, space="DRAM") as dpool,
            tc.tile_pool(name="const", bufs=1) as cpool,
            tc.tile_pool(name="stat", bufs=6) as spool,
            tc.tile_pool(name="work", bufs=2) as wpool,
            tc.tile_pool(name="gbuf", bufs=5) as gpool,
            tc.tile_pool(name="pbuf", bufs=2) as ppool,
            tc.tile_pool(name="small", bufs=3) as mpool,
            tc.tile_pool(name="idx", bufs=18) as ipool,
            tc.tile_pool(name="psA", bufs=2, space="PSUM") as psA,
            tc.tile_pool(name="psT", bufs=2, space="PSUM") as psT,
            tc.tile_pool(name="psB", bufs=2, space="PSUM") as psB,
        ):
            # ---- resident constants ----
            ident = cpool.tile([128, 128], F32, tag="ident")
            make_identity(nc, ident[:])
            ident16 = cpool.tile([128, 128], FP16, tag="ident16")
            make_identity(nc, ident16[:])
            const_sb = cpool.tile([128, 6, 16], F32, tag="consts")
            nc.sync.dma_start(out=const_sb[:], in_=consts)
            lw_c = const_sb[:, 0, :]
            lwm1_c = const_sb[:, 1, :]
            lhm1_c = const_sb[:, 2, :]
            start_c = const_sb[:, 3, :]
            lwm15_c = const_sb[:, 4, :]
            lhm15_c = const_sb[:, 5, :]

            wrap_sb = cpool.tile([128, 136], F32, tag="wrapc")
            nc.sync.dma_start(out=wrap_sb[:], in_=wrapc)
            s128_c = wrap_sb[:, 0:128]
            maskc_c = wrap_sb[:, 128:136]

            wa_sb = []
            for i, (o, kk) in enumerate(KA_CH):
                t = cpool.tile([kk, 96], F32, tag=f"wa{i}", name=f"wa{i}")
                nc.sync.dma_start(out=t[:], in_=waug[o:o + kk, :])
                wa_sb.append(t)
            wv_sb = []
            for i, (o, kk) in enumerate(KV_CH):
                t = cpool.tile([kk, 64], BF16, tag=f"wv{i}", name=f"wv{i}")
                nc.sync.dma_start(out=t[:], in_=wvaug[o:o + kk, :])
                wv_sb.append(t)
            wout_sb = cpool.tile([64, 256], F32, tag="wout")
            nc.sync.dma_start(out=wout_sb[:], in_=wout)

            bp05 = cpool.tile([128, 1], F32, tag="bp05")
            nc.vector.memset(bp05[:], 0.5)
            bm05 = cpool.tile([128, 1], F32, tag="bm05")
            nc.vector.memset(bm05[:], -0.5)

            ni_reg = nc.gpsimd.to_reg(2048)

            # ---- DRAM scratch ----
            vtab = [dpool.tile([NQP, 32], FP16, tag=f"vtab{h}", name=f"vtab{h}")
                    for h in range(2)]
            tab = [dpool.tile([NQP, 128], FP16, tag=f"tab{h}", name=f"tab{h}")
                   for h in range(2)]
            tabv = [tab[h][:] for h in range(2)]

            # ---- pipelined P2/P3/P4: emission order = engine-queue order,
            # so chunk c+1's projections/indices are emitted before chunk c's
            # gather consumers to keep every engine fed while Pool gathers. ----
            i16s = {}    # (chunk_t0, i, h) -> i16 tile
            w4hs = {}    # chunk_t0 -> [w4h_h0, w4h_h1]

            def emit_p2(t0):
                nt = min(CHUNK, NT - t0)
                proj = wpool.tile([128, CHUNK, 96], F32, tag="proj")
                qas = []
                for j, (o, kk) in enumerate(KA_CH):
                    qa = spool.tile([128, CHUNK * 128], F32, tag="ld",
                                    name=f"qab{j}")
                    nc.scalar.dma_start(
                        out=qa[:kk, :nt * 128],
                        in_=qaug[o:o + kk, t0 * 128:(t0 + nt) * 128])
                    qas.append(qa)
                for i in range(nt):
                    psp = psA.tile([128, 96], F32, tag="ps_a")
                    for j, (o, kk) in enumerate(KA_CH):
                        nc.tensor.matmul(
                            out=psp[:], lhsT=qas[j][:kk, i * 128:(i + 1) * 128],
                            rhs=wa_sb[j][:],
                            start=(j == 0), stop=(j == len(KA_CH) - 1))
                    nc.scalar.copy(out=proj[:, i, :], in_=psp[:])

                w4b = wpool.tile([128, CHUNK, 2, 16, 4], F32, tag="w4b")
                for h in range(2):
                    c0 = h * 48
                    px = proj[:, :nt, c0 + 0:c0 + 16]
                    py = proj[:, :nt, c0 + 16:c0 + 32]
                    att = proj[:, :nt, c0 + 32:c0 + 48]

                    def bc(c16):
                        return c16.unsqueeze(1).to_broadcast([128, nt, 16])

                    # softmax over 16 (l,p)
                    e = wpool.tile([128, CHUNK, 16], F32, tag=f"e{h}")
                    nc.scalar.activation(out=e[:, :nt, :], in_=att, func=ACT.Exp)
                    ssum = mpool.tile([128, CHUNK], F32, tag=f"ss{h}")
                    nc.vector.tensor_reduce(out=ssum[:, :nt], in_=e[:, :nt, :],
                                            axis=AX.X, op=OP.add)
                    rinv = mpool.tile([128, CHUNK], F32, tag=f"ri{h}")
                    nc.vector.reciprocal(out=rinv[:, :nt], in_=ssum[:, :nt])
                    nc.vector.tensor_tensor(
                        out=e[:, :nt, :], in0=e[:, :nt, :],
                        in1=rinv[:, :nt].unsqueeze(2).to_broadcast([128, nt, 16]),
                        op=OP.mult)

                    # floor/clip
                    x0f = wpool.tile([128, CHUNK, 16], F32, tag=f"x0{h}")
                    y0f = wpool.tile([128, CHUNK, 16], F32, tag=f"y0{h}")
                    nc.vector.tensor_scalar(out=x0f[:, :nt, :], in0=px,
                                            scalar1=MAGIC, scalar2=MAGIC,
                                            op0=OP.add, op1=OP.subtract)
                    nc.vector.tensor_scalar(out=y0f[:, :nt, :], in0=py,
                                            scalar1=MAGIC, scalar2=MAGIC,
                                            op0=OP.add, op1=OP.subtract)
                    xb = x0f
                    yb = y0f
                    nc.vector.tensor_scalar_max(out=xb[:, :nt, :],
                                                in0=xb[:, :nt, :], scalar1=0.0)
                    nc.vector.tensor_tensor(out=xb[:, :nt, :], in0=xb[:, :nt, :],
                                            in1=bc(lwm1_c), op=OP.min)
                    nc.vector.tensor_scalar_max(out=yb[:, :nt, :],
                                                in0=yb[:, :nt, :], scalar1=0.0)
                    nc.vector.tensor_tensor(out=yb[:, :nt, :], in0=yb[:, :nt, :],
                                            in1=bc(lhm1_c), op=OP.min)

                    # slot weights: wq[axis, pm] = relu(1 - |d +/- 0.5|)
                    d2 = wpool.tile([128, CHUNK, 2, 16], F32, tag=f"d2{h}")
                    nc.vector.tensor_tensor(out=d2[:, :nt, 0, :], in0=px,
                                            in1=xb[:, :nt, :], op=OP.subtract)
                    nc.vector.tensor_tensor(out=d2[:, :nt, 1, :], in0=py,
                                            in1=yb[:, :nt, :], op=OP.subtract)
                    wq = wpool.tile([128, CHUNK, 2, 2, 16], F32, tag=f"wq{h}")
                    nc.scalar.activation(out=wq[:, :nt, :, 0, :],
                                         in_=d2[:, :nt, :, :],
                                         func=ACT.Abs, bias=bp05[:])
                    nc.scalar.activation(out=wq[:, :nt, :, 1, :],
                                         in_=d2[:, :nt, :, :],
                                         func=ACT.Abs, bias=bm05[:])
                    nc.scalar.activation(out=wq[:, :nt, :, :, :],
                                         in_=wq[:, :nt, :, :, :],
                                         func=ACT.Relu, scale=-1.0, bias=1.0)

                    mx = wpool.tile([128, CHUNK, 16], F32, tag=f"mx{h}")
                    my = wpool.tile([128, CHUNK, 16], F32, tag=f"my{h}")
                    nc.vector.tensor_tensor(out=mx[:, :nt, :], in0=px,
                                            in1=bc(lwm15_c), op=OP.is_lt)
                    nc.vector.tensor_tensor(out=my[:, :nt, :], in0=py,
                                            in1=bc(lhm15_c), op=OP.is_lt)
                    nc.vector.tensor_tensor(out=wq[:, :nt, 0, 1, :],
                                            in0=wq[:, :nt, 0, 1, :],
                                            in1=mx[:, :nt, :], op=OP.mult)
                    nc.vector.tensor_tensor(out=wq[:, :nt, 1, 1, :],
                                            in0=wq[:, :nt, 1, 1, :],
                                            in1=my[:, :nt, :], op=OP.mult)
                    nc.vector.tensor_tensor(out=wq[:, :nt, 1, 0, :],
                                            in0=wq[:, :nt, 1, 0, :],
                                            in1=e[:, :nt, :], op=OP.mult)
                    nc.vector.tensor_tensor(out=wq[:, :nt, 1, 1, :],
                                            in0=wq[:, :nt, 1, 1, :],
                                            in1=e[:, :nt, :], op=OP.mult)

                    # w4[q, (h, lp, slot)]  slot = dy*2+dx
                    for s, (ydx, xdx) in enumerate(
                            ((0, 0), (0, 1), (1, 0), (1, 1))):
                        nc.vector.tensor_tensor(out=w4b[:, :nt, h, :, s],
                                                in0=wq[:, :nt, 1, ydx, :],
                                                in1=wq[:, :nt, 0, xdx, :],
                                                op=OP.mult)

                    # idx f32 = start + yb*lw + xb
                    nc.vector.tensor_tensor(out=yb[:, :nt, :], in0=yb[:, :nt, :],
                                            in1=bc(lw_c), op=OP.mult)
                    nc.vector.tensor_tensor(out=xb[:, :nt, :], in0=xb[:, :nt, :],
                                            in1=yb[:, :nt, :], op=OP.add)
                    nc.vector.tensor_tensor(out=xb[:, :nt, :], in0=xb[:, :nt, :],
                                            in1=bc(start_c), op=OP.add)
                    # idx wrap per tile: sample order i = lp*128 + q, wrap
                    # slot [i%16, i//16] = [q%16, lp*8 + q//16].  R[q, lp, c]
                    # = idx[q, lp] * (q//16 == c); psw[m, (lp, c)] =
                    # sum_q (q%16 == m%16) R[q, lp, c] = idx[c*16+m%16, lp]
                    # -> 8 replicas across partitions (sim reads 0-15, HW
                    # queue k reads 32k..32k+32).
                    for i in range(nt):
                        rw = mpool.tile([128, 16, 8], F32, tag="rw")
                        nc.vector.tensor_tensor(
                            out=rw[:],
                            in0=xb[:, i, :].unsqueeze(2).to_broadcast(
                                [128, 16, 8]),
                            in1=maskc_c.unsqueeze(1).to_broadcast([128, 16, 8]),
                            op=OP.mult)
                        psw = psT.tile([128, 128], F32, tag="ps_t")
                        nc.tensor.matmul(
                            out=psw[:], lhsT=s128_c,
                            rhs=rw[:].rearrange("p l c -> p (l c)"),
                            start=True, stop=True)
                        i16 = ipool.tile([128, 128], I16, tag="i16")
                        nc.vector.tensor_copy(out=i16[:], in_=psw[:])
                        i16s[(t0, i, h)] = i16
                w4hs[t0] = w4b

            def emit_p3(t0):
                nt = min(CHUNK, NT - t0)
                w4b = w4hs.pop(t0)
                msda = wpool.tile([128, CHUNK, 64], F32, tag="msda")
                for i in range(nt):
                    t = t0 + i
                    # g[q, h, lp, s, ch] (sample order i = lp*128 + q); the
                    # two per-head gathers fill halves of one tile so the
                    # multiply/reduce run as single double-width DVE ops.
                    g = gpool.tile([128, 2, 16, 4, 32], FP16, tag="g")
                    for h in range(2):
                        i16 = i16s.pop((t0, i, h))
                        nc.gpsimd.dma_gather(
                            out_ap=g[:, h].rearrange("p a b c -> p a (b c)"),
                            in_ap=tabv[h], idxs_ap=i16[:],
                            num_idxs=2048, num_idxs_reg=ni_reg,
                            elem_size=128, single_packet=False,
                            queue_num=(t * 2 + h) % 4)
                    p16 = ppool.tile([128, 2, 16, 4, 32], FP16, tag="p16")
                    nc.vector.tensor_tensor(
                        out=p16[:], in0=g[:],
                        in1=w4b[:, i, :, :, :].unsqueeze(4).to_broadcast(
                            [128, 2, 16, 4, 32]),
                        op=OP.mult)
                    # slot sum: 3 contiguous-run adds -> ssum[q, h, lp, ch]
                    ssum = ppool.tile([128, 2, 16, 32], FP16, tag="ssum")
                    nc.vector.tensor_tensor(
                        out=ssum[:], in0=p16[:, :, :, 0, :],
                        in1=p16[:, :, :, 1, :], op=OP.add)
                    nc.vector.tensor_tensor(
                        out=ssum[:], in0=ssum[:],
                        in1=p16[:, :, :, 2, :], op=OP.add)
                    nc.vector.tensor_tensor(
                        out=ssum[:], in0=ssum[:],
                        in1=p16[:, :, :, 3, :], op=OP.add)
                    # lp sum: contiguous halving tree (fp16) -> fp32 msda
                    nc.vector.tensor_tensor(
                        out=ssum[:, :, :8, :], in0=ssum[:, :, :8, :],
                        in1=ssum[:, :, 8:, :], op=OP.add)
                    nc.vector.tensor_tensor(
                        out=ssum[:, :, :4, :], in0=ssum[:, :, :4, :],
                        in1=ssum[:, :, 4:8, :], op=OP.add)
                    nc.vector.tensor_tensor(
                        out=ssum[:, :, :2, :], in0=ssum[:, :, :2, :],
                        in1=ssum[:, :, 2:4, :], op=OP.add)
                    nc.vector.tensor_tensor(
                        out=msda[:, i, :].rearrange("p (a c) -> p a c", a=2),
                        in0=ssum[:, :, 0, :], in1=ssum[:, :, 1, :], op=OP.add)
                return msda

            def emit_p4(t0, msda):
                nt = min(CHUNK, NT - t0)
                osb = wpool.tile([128, CHUNK, 256], F32, tag="osb")
                for i in range(nt):
                    psb = psB.tile([64, 128], F32, tag="ps_b")
                    nc.tensor.transpose(out=psb[:], in_=msda[:, i, :],
                                        identity=ident[:])
                    mT = mpool.tile([64, 128], F32, tag="mT")
                    nc.scalar.copy(out=mT[:], in_=psb[:])
                    pso = psB.tile([128, 256], F32, tag="ps_o")
                    nc.tensor.matmul(out=pso[:], lhsT=mT[:], rhs=wout_sb[:],
                                     start=True, stop=True)
                    nc.scalar.copy(out=osb[:, i, :], in_=pso[:])
                nc.scalar.dma_start(
                    out=outp[t0 * 128:(t0 + nt) * 128, :].rearrange(
                        "(t q) c -> q t c", q=128),
                    in_=osb[:, :nt, :])

            emit_p2(0)
            # ---- P1: value projection -> vtab ----
            def emit_p1b(lv):
                lh, lw = SPATIAL[lv]
                s0, hw = STARTS[lv], HWS[lv]
                for h in range(2):
                    for s, sh in enumerate((0, 1, lw, lw + 1)):
                        nc.sync.dma_start(
                            out=tab[h][s0:s0 + hw, s * 32:(s + 1) * 32],
                            in_=vtab[h][s0 + sh:s0 + sh + hw, :])

            t0 = 0
            while t0 < NT:
                nt = min(CHUNK, NT - t0)
                vas = []
                for i, (o, kk) in enumerate(KV_CH):
                    va = spool.tile([128, CHUNK * 128], BF16, tag="vld",
                                    name=f"vab{i}")
                    nc.sync.dma_start(
                        out=va[:kk, :nt * 128],
                        in_=vaug[o:o + kk, t0 * 128:(t0 + nt) * 128])
                    vas.append(va)
                vsb = mpool.tile([128, CHUNK, 64], FP16, tag="vsb")
                for i in range(nt):
                    psv = psA.tile([128, 64], F32, tag="ps_a")
                    for j, (o, kk) in enumerate(KV_CH):
                        nc.tensor.matmul(
                            out=psv[:], lhsT=vas[j][:kk, i * 128:(i + 1) * 128],
                            rhs=wv_sb[j][:],
                            start=(j == 0), stop=(j == len(KV_CH) - 1))
                    nc.scalar.copy(out=vsb[:, i, :], in_=psv[:])
                for h in range(2):
                    nc.sync.dma_start(
                        out=vtab[h][t0 * 128:(t0 + nt) * 128, :].rearrange(
                            "(t q) c -> q t c", q=128),
                        in_=vsb[:, :nt, h * 32:(h + 1) * 32])
                t0 += nt
                # interleave patch-table builds: emit each level's slot DMAs
                # as soon as the vtab rows it reads are stored
                if t0 == 72:
                    emit_p1b(0)
                elif t0 == 88:
                    emit_p1b(1)
            emit_p1b(2)
            emit_p1b(3)

            chunk_starts = list(range(0, NT, CHUNK))
            for ci, t0 in enumerate(chunk_starts):
                if ci + 1 < len(chunk_starts):
                    emit_p2(chunk_starts[ci + 1])
                msda = emit_p3(t0)
                emit_p4(t0, msda)
    nc.compile()
    return nc


def host_prep(inputs):
    q = np.asarray(inputs["query"], np.float32)
    ref = np.asarray(inputs["reference_points"], np.float32)
    val = np.asarray(inputs["value"], np.float32)
    W_off = np.asarray(inputs["W_off"], np.float32)
    b_off = np.asarray(inputs["b_off"], np.float32)
    W_attn = np.asarray(inputs["W_attn"], np.float32)
    b_attn = np.asarray(inputs["b_attn"], np.float32)
    W_val = np.asarray(inputs["W_val"], np.float32)
    b_val = np.asarray(inputs["b_val"], np.float32)
    W_out = np.asarray(inputs["W_out"], np.float32)

    lh = np.array([s[0] for s in SPATIAL], np.float32)
    lw = np.array([s[1] for s in SPATIAL], np.float32)

    qaug = np.zeros((B, KA, NQP), np.float32)
    for b in range(B):
        qaug[b, :DM, :NQ] = q[b].T
        rx = ref[b, :, :, 0] * lw[None, :] - 1.0   # px' = px - 0.5
        ry = ref[b, :, :, 1] * lh[None, :] - 1.0
        qaug[b, DM:DM + 16, :NQ] = np.repeat(rx, P, axis=1).T
        qaug[b, DM + 16:DM + 32, :NQ] = np.repeat(ry, P, axis=1).T
        qaug[b, DM + 32, :] = 1.0

    vaug = np.zeros((B, KV, NQP), np.float32)
    for b in range(B):
        vaug[b, :DM, :NV] = val[b].T
        vaug[b, DM, :] = 1.0

    W_off_r = W_off.reshape(DM, H, L, P, 2)
    b_off_r = b_off.reshape(H, L, P, 2)
    W_attn_r = W_attn.reshape(DM, H, L, P)
    b_attn_r = b_attn.reshape(H, L, P)

    waug_all = np.zeros((H, KA, 48), np.float32)
    for h in range(H):
        waug_all[h, :DM, 0:16] = W_off_r[:, h, :, :, 0].reshape(DM, 16)
        waug_all[h, :DM, 16:32] = W_off_r[:, h, :, :, 1].reshape(DM, 16)
        waug_all[h, :DM, 32:48] = W_attn_r[:, h].reshape(DM, 16)
        waug_all[h, DM + 32, 0:16] = b_off_r[h, :, :, 0].reshape(16)
        waug_all[h, DM + 32, 16:32] = b_off_r[h, :, :, 1].reshape(16)
        waug_all[h, DM + 32, 32:48] = b_attn_r[h].reshape(16)
        for j in range(16):
            waug_all[h, DM + j, j] = 1.0
            waug_all[h, DM + 16 + j, 16 + j] = 1.0

    W_val_r = W_val.reshape(DM, H, HD)
    b_val_r = b_val.reshape(H, HD)

    consts = np.zeros((128, 6, 16), np.float32)
    consts[:, 0, :] = np.repeat(lw, P)[None, :]
    consts[:, 1, :] = np.repeat(lw - 1.0, P)[None, :]
    consts[:, 2, :] = np.repeat(lh - 1.0, P)[None, :]
    consts[:, 3, :] = np.repeat(np.array(STARTS, np.float32), P)[None, :]
    consts[:, 4, :] = np.repeat(lw - 1.5, P)[None, :]
    consts[:, 5, :] = np.repeat(lh - 1.5, P)[None, :]

    wrapc = np.zeros((128, 136), np.float32)
    for q in range(128):
        for m in range(128):
            if q % 16 == m % 16:
                wrapc[q, m] = 1.0
        wrapc[q, 128 + q // 16] = 1.0

    in_maps = []
    for c in range(N_CORES):
        b = c // 4
        h0 = 2 * (c % 4)
        waug = np.concatenate([waug_all[h0], waug_all[h0 + 1]], axis=1)
        wv = np.zeros((KV, 64), np.float32)
        wv[:DM, 0:32] = W_val_r[:, h0, :]
        wv[:DM, 32:64] = W_val_r[:, h0 + 1, :]
        wv[DM, 0:32] = b_val_r[h0]
        wv[DM, 32:64] = b_val_r[h0 + 1]
        wo = np.ascontiguousarray(
            W_out.reshape(H, HD, DM)[h0:h0 + 2].reshape(64, DM))
        in_maps.append({
            "qaug": np.ascontiguousarray(qaug[b]),
            "vaug": np.ascontiguousarray(vaug[b]).astype(ml_dtypes.bfloat16),
            "waug": np.ascontiguousarray(waug),
            "wvaug": wv.astype(ml_dtypes.bfloat16),
            "wout": wo,
            "consts": consts,
            "wrapc": wrapc,
        })
    return in_maps


_NC_CACHE = None


def kernel(**inputs) -> np.ndarray:
    global _NC_CACHE
    in_maps = host_prep(inputs)
    if _NC_CACHE is None:
        _NC_CACHE = build_module()
    nc = _NC_CACHE
    res = run_bass_kernel_spmd(nc, in_maps, core_ids=list(range(N_CORES)))
    b_out = np.asarray(inputs["b_out"], np.float32)
    out = np.zeros((B, NQ, DM), np.float32)
    for c in range(N_CORES):
        out[c // 4] += res.results[c]["outp"][:NQ, :]
    out += b_out[None, None, :]
    return out


if __name__ == "__main__":
    import reference

    inputs = {k: np.asarray(v) for k, v in reference.setup_inputs().items()}
    got = kernel(**inputs)
    exp = np.asarray(reference.reference(**inputs))
    err = np.abs(got - exp)
    rel = np.linalg.norm(got - exp) / np.linalg.norm(exp)
    print("abs max err:", err.max(), "rel:", rel)



# revision 36
# speedup vs baseline: 1.9649x; 1.2435x over previous
"""Multi-Scale Deformable Attention (DigitDETR encoder layer) on 8 TRN2 cores.

Sharding: 16 (batch, head) pairs over 8 cores -> each core handles one batch
and two consecutive heads (data-parallel over B, tensor-parallel over H).
Each core computes a partial output  msda(b, h0..h1) @ W_out[h-rows]; the host
sums the 4 partials per batch and adds b_out during unsharding.

Per-core device pipeline (Tile framework):
  P1  value projection (PE) -> v[h] in DRAM -> column-major x-duplicated
      "patch table" per head: row r = pixel (x, y), content = [v(y,x), v(y,x+1)]
      (64 f32).  Rows r, r+1 are y-neighbors, so one 512B read at row r
      yields the full 2x2 bilinear patch (elem_size=128, elem_step=64).
  P2  fused projection matmul (query_T | ref*size-1.0 | ones) @ W_aug gives
      px' (= px-0.5), py', attn logits; softmax; floor via +/-2^23 magic;
      clip; slot weights relu(1-|d±.5|) with border masks; idx f32.
  P3  gather via dma_gather with sample order i = lp*128+q so G lands
      q-partitioned [q, lp, 4, 32]; idx wrap [q%16, lp*8+q//16] built by a
      masked-replicate matmul (R = idx*maskC; psw = S128 @ R -> 8 replicas
      for sim partitions 0-15 + all 4 SWDGE queue core pairs); gathers
      round-robin queues 0-3 so 4 Q7 pairs generate descriptors in
      parallel; DVE product with w4 read straight from P2 (no DRAM
      bounce); 2 DVE reduces (slots, then lp) -> msda[q, 64] in SBUF.
  P4  PE-transpose msda, out-projection matmul -> rows.
"""

import ml_dtypes
import numpy as np

import concourse.bass as bass
import concourse.bacc as bacc
import concourse.mybir as mybir
import concourse.tile as tile
from concourse.bass_utils import run_bass_kernel_spmd
from concourse.masks import make_identity

F32 = mybir.dt.float32
FP16 = mybir.dt.float16
BF16 = mybir.dt.bfloat16
I16 = mybir.dt.int16
AX = mybir.AxisListType
OP = mybir.AluOpType
ACT = mybir.ActivationFunctionType

# ---- static problem config ----
SPATIAL = ((76, 114), (38, 57), (19, 29), (10, 15))  # (lh, lw)
HWS = [h * w for h, w in SPATIAL]
STARTS = [0, 8664, 10830, 11381]
NV = 11531
B, H, L, P, DM, HD = 2, 8, 4, 4, 256, 32
NQ = NV
NT = 91
NQP = NT * 128          # 11648
KA = DM + 2 * L * P + 1  # 289
KV = DM + 1              # 257
MAGIC = 12582912.0       # 1.5 * 2^23
CHUNK = 8

N_CORES = 8


def _kchunks(k):
    out, o = [], 0
    while o < k:
        kk = min(128, k - o)
        out.append((o, kk))
        o += kk
    return out


KA_CH = _kchunks(KA)
KV_CH = _kchunks(KV)


def build_module(reps=1):
    nc = bacc.Bacc("TRN2", target_bir_lowering=False, debug=False,
                   enable_asserts=False, num_devices=N_CORES,
                   num_swdge_queues=4, dynamic_dma_scratch_size=40960)

    qaug = nc.dram_tensor("qaug", [KA, NQP], F32, kind="ExternalInput").ap()
    vaug = nc.dram_tensor("vaug", [KV, NQP], BF16, kind="ExternalInput").ap()
    waug = nc.dram_tensor("waug", [KA, 96], F32, kind="ExternalInput").ap()
    wvaug = nc.dram_tensor("wvaug", [KV, 64], BF16, kind="ExternalInput").ap()
    wout = nc.dram_tensor("wout", [64, 256], F32, kind="ExternalInput").ap()
    # rows: 0=lw, 1=lw-1, 2=lh-1, 3=start, 4=lw-1.5, 5=lh-1.5
    consts = nc.dram_tensor("consts", [128, 6, 16], F32, kind="ExternalInput").ap()
    # wrapc[:, :128] = S128[q, m] = (q%16 == m%16); [:, 128:136] = maskC[q, c]
    # = (q//16 == c)
    wrapc = nc.dram_tensor("wrapc", [128, 136], F32, kind="ExternalInput").ap()
    outp = nc.dram_tensor("outp", [NQP, 256], F32, kind="ExternalOutput").ap()

    with tile.TileContext(nc) as tc:
        with (
            tc.tile_pool(name="dram", bufs=1, space="DRAM") as dpool,
            tc.tile_pool(name="const", bufs=1) as cpool,
            tc.tile_pool(name="stat", bufs=6) as spool,
            tc.tile_pool(name="work", bufs=2) as wpool,
            tc.tile_pool(name="gbuf", bufs=5) as gpool,
            tc.tile_pool(name="pbuf", bufs=2) as ppool,
            tc.tile_pool(name="small", bufs=3) as mpool,
            tc.tile_pool(name="idx", bufs=18) as ipool,
            tc.tile_pool(name="psA", bufs=2, space="PSUM") as psA,
            tc.tile_pool(name="psT", bufs=2, space="PSUM") as psT,
            tc.tile_pool(name="psB", bufs=2, space="PSUM") as psB,
        ):
            # ---- resident constants ----
            ident = cpool.tile([128, 128], F32, tag="ident")
            make_identity(nc, ident[:])
            ident16 = cpool.tile([128, 128], FP16, tag="ident16")
            make_identity(nc, ident16[:])
            const_sb = cpool.tile([128, 6, 16], F32, tag="consts")
            nc.sync.dma_start(out=const_sb[:], in_=consts)
            lw_c = const_sb[:, 0, :]
            lwm1_c = const_sb[:, 1, :]
            lhm1_c = const_sb[:, 2, :]
            start_c = const_sb[:, 3, :]
            lwm15_c = const_sb[:, 4, :]
            lhm15_c = const_sb[:, 5, :]

            wrap_sb = cpool.tile([128, 136], F32, tag="wrapc")
            nc.sync.dma_start(out=wrap_sb[:], in_=wrapc)
            s128_c = wrap_sb[:, 0:128]
            maskc_c = wrap_sb[:, 128:136]

            wa_sb = []
            for i, (o, kk) in enumerate(KA_CH):
                t = cpool.tile([kk, 96], F32, tag=f"wa{i}", name=f"wa{i}")
                nc.sync.dma_start(out=t[:], in_=waug[o:o + kk, :])
                wa_sb.append(t)
            wv_sb = []
            for i, (o, kk) in enumerate(KV_CH):
                t = cpool.tile([kk, 64], BF16, tag=f"wv{i}", name=f"wv{i}")
                nc.sync.dma_start(out=t[:], in_=wvaug[o:o + kk, :])
                wv_sb.append(t)
            wout_sb = cpool.tile([64, 256], F32, tag="wout")
            nc.sync.dma_start(out=wout_sb[:], in_=wout)

            bp05 = cpool.tile([128, 1], F32, tag="bp05")
            nc.vector.memset(bp05[:], 0.5)
            bm05 = cpool.tile([128, 1], F32, tag="bm05")
            nc.vector.memset(bm05[:], -0.5)

            ni_reg = nc.gpsimd.to_reg(2048)

            # ---- DRAM scratch ----
            vtab = [dpool.tile([NQP, 32], FP16, tag=f"vtab{h}", name=f"vtab{h}")
                    for h in range(2)]
            tab = [dpool.tile([NQP, 128], FP16, tag=f"tab{h}", name=f"tab{h}")
                   for h in range(2)]
            tabv = [tab[h][:] for h in range(2)]

            # ---- pipelined P2/P3/P4: emission order = engine-queue order,
            # so chunk c+1's projections/indices are emitted before chunk c's
            # gather consumers to keep every engine fed while Pool gathers. ----
            i16s = {}    # (chunk_t0, i, h) -> i16 tile
            w4hs = {}    # chunk_t0 -> [w4h_h0, w4h_h1]

            def emit_p2(t0):
                nt = min(CHUNK, NT - t0)
                pxh = [wpool.tile([128, CHUNK, 16], F32, tag=f"px{h}",
                                  name=f"px{h}") for h in range(2)]
                pyh = [wpool.tile([128, CHUNK, 16], F32, tag=f"py{h}",
                                  name=f"py{h}") for h in range(2)]
                ath = [wpool.tile([128, CHUNK, 16], F32, tag=f"at{h}",
                                  name=f"at{h}") for h in range(2)]
                qas = []
                for j, (o, kk) in enumerate(KA_CH):
                    qa = spool.tile([128, CHUNK * 128], F32, tag="ld",
                                    name=f"qab{j}")
                    nc.scalar.dma_start(
                        out=qa[:kk, :nt * 128],
                        in_=qaug[o:o + kk, t0 * 128:(t0 + nt) * 128])
                    qas.append(qa)
                for i in range(nt):
                    psp = psA.tile([128, 96], F32, tag="ps_a")
                    for j, (o, kk) in enumerate(KA_CH):
                        nc.tensor.matmul(
                            out=psp[:], lhsT=qas[j][:kk, i * 128:(i + 1) * 128],
                            rhs=wa_sb[j][:],
                            start=(j == 0), stop=(j == len(KA_CH) - 1))
                    # pack px/py/att per head contiguously so every DVE op
                    # below reads full-run operands
                    for h2 in range(2):
                        c0 = h2 * 48
                        nc.scalar.copy(out=pxh[h2][:, i, :],
                                       in_=psp[:, c0:c0 + 16])
                        nc.scalar.copy(out=pyh[h2][:, i, :],
                                       in_=psp[:, c0 + 16:c0 + 32])
                        nc.scalar.copy(out=ath[h2][:, i, :],
                                       in_=psp[:, c0 + 32:c0 + 48])

                w4b = wpool.tile([128, CHUNK, 2, 16, 4], F32, tag="w4b")
                xbs = [None, None]
                for h in range(2):
                    px = pxh[h][:, :nt, :]
                    py = pyh[h][:, :nt, :]
                    att = ath[h][:, :nt, :]

                    def bc(c16):
                        return c16.unsqueeze(1).to_broadcast([128, nt, 16])

                    # softmax over 16 (l,p)
                    e = wpool.tile([128, CHUNK, 16], F32, tag=f"e{h}")
                    nc.scalar.activation(out=e[:, :nt, :], in_=att, func=ACT.Exp)
                    ssum = mpool.tile([128, CHUNK], F32, tag=f"ss{h}")
                    nc.vector.tensor_reduce(out=ssum[:, :nt], in_=e[:, :nt, :],
                                            axis=AX.X, op=OP.add)
                    rinv = mpool.tile([128, CHUNK], F32, tag=f"ri{h}")
                    nc.vector.reciprocal(out=rinv[:, :nt], in_=ssum[:, :nt])
                    nc.vector.tensor_tensor(
                        out=e[:, :nt, :], in0=e[:, :nt, :],
                        in1=rinv[:, :nt].unsqueeze(2).to_broadcast([128, nt, 16]),
                        op=OP.mult)

                    # floor/clip
                    x0f = wpool.tile([128, CHUNK, 16], F32, tag=f"x0{h}")
                    y0f = wpool.tile([128, CHUNK, 16], F32, tag=f"y0{h}")
                    nc.vector.tensor_scalar(out=x0f[:, :nt, :], in0=px,
                                            scalar1=MAGIC, scalar2=MAGIC,
                                            op0=OP.add, op1=OP.subtract)
                    nc.vector.tensor_scalar(out=y0f[:, :nt, :], in0=py,
                                            scalar1=MAGIC, scalar2=MAGIC,
                                            op0=OP.add, op1=OP.subtract)
                    xb = x0f
                    yb = y0f
                    nc.vector.tensor_scalar_max(out=xb[:, :nt, :],
                                                in0=xb[:, :nt, :], scalar1=0.0)
                    nc.vector.tensor_tensor(out=xb[:, :nt, :], in0=xb[:, :nt, :],
                                            in1=bc(lwm1_c), op=OP.min)
                    nc.vector.tensor_scalar_max(out=yb[:, :nt, :],
                                                in0=yb[:, :nt, :], scalar1=0.0)
                    nc.vector.tensor_tensor(out=yb[:, :nt, :], in0=yb[:, :nt, :],
                                            in1=bc(lhm1_c), op=OP.min)

                    # slot weights: wq[axis, pm] = relu(1 - |d +/- 0.5|)
                    d2 = wpool.tile([128, CHUNK, 2, 16], F32, tag=f"d2{h}")
                    nc.vector.tensor_tensor(out=d2[:, :nt, 0, :], in0=px,
                                            in1=xb[:, :nt, :], op=OP.subtract)
                    nc.vector.tensor_tensor(out=d2[:, :nt, 1, :], in0=py,
                                            in1=yb[:, :nt, :], op=OP.subtract)
                    wq = wpool.tile([128, CHUNK, 2, 2, 16], F32, tag=f"wq{h}")
                    nc.scalar.activation(out=wq[:, :nt, :, 0, :],
                                         in_=d2[:, :nt, :, :],
                                         func=ACT.Abs, bias=bp05[:])
                    nc.scalar.activation(out=wq[:, :nt, :, 1, :],
                                         in_=d2[:, :nt, :, :],
                                         func=ACT.Abs, bias=bm05[:])
                    nc.scalar.activation(out=wq[:, :nt, :, :, :],
                                         in_=wq[:, :nt, :, :, :],
                                         func=ACT.Relu, scale=-1.0, bias=1.0)

                    mx = wpool.tile([128, CHUNK, 16], F32, tag=f"mx{h}")
                    my = wpool.tile([128, CHUNK, 16], F32, tag=f"my{h}")
                    nc.vector.tensor_tensor(out=mx[:, :nt, :], in0=px,
                                            in1=bc(lwm15_c), op=OP.is_lt)
                    nc.vector.tensor_tensor(out=my[:, :nt, :], in0=py,
                                            in1=bc(lhm15_c), op=OP.is_lt)
                    nc.vector.tensor_tensor(out=wq[:, :nt, 0, 1, :],
                                            in0=wq[:, :nt, 0, 1, :],
                                            in1=mx[:, :nt, :], op=OP.mult)
                    nc.vector.tensor_tensor(out=wq[:, :nt, 1, 1, :],
                                            in0=wq[:, :nt, 1, 1, :],
                                            in1=my[:, :nt, :], op=OP.mult)
                    nc.vector.tensor_tensor(out=wq[:, :nt, 1, 0, :],
                                            in0=wq[:, :nt, 1, 0, :],
                                            in1=e[:, :nt, :], op=OP.mult)
                    nc.vector.tensor_tensor(out=wq[:, :nt, 1, 1, :],
                                            in0=wq[:, :nt, 1, 1, :],
                                            in1=e[:, :nt, :], op=OP.mult)

                    # w4[q, (h, lp, slot)]  slot = dy*2+dx
                    for s, (ydx, xdx) in enumerate(
                            ((0, 0), (0, 1), (1, 0), (1, 1))):
                        nc.vector.tensor_tensor(out=w4b[:, :nt, h, :, s],
                                                in0=wq[:, :nt, 1, ydx, :],
                                                in1=wq[:, :nt, 0, xdx, :],
                                                op=OP.mult)

                    # idx f32 = start + yb*lw + xb
                    nc.vector.tensor_tensor(out=yb[:, :nt, :], in0=yb[:, :nt, :],
                                            in1=bc(lw_c), op=OP.mult)
                    nc.vector.tensor_tensor(out=xb[:, :nt, :], in0=xb[:, :nt, :],
                                            in1=yb[:, :nt, :], op=OP.add)
                    nc.vector.tensor_tensor(out=xb[:, :nt, :], in0=xb[:, :nt, :],
                                            in1=bc(start_c), op=OP.add)
                    xbs[h] = xb
                # idx wrap per tile, both heads in one matmul: sample order
                # i = lp*128 + q, wrap slot [i%16, i//16] = [q%16, lp*8 +
                # q//16].  rwb[q, h, lp, c] = idx_h[q, lp] * (q//16 == c);
                # psw[m, (h, lp, c)] = sum_q (q%16 == m%16) rwb =
                # idx_h[c*16+m%16, lp] -> 8 replicas across partitions (sim
                # reads 0-15, HW queue k reads 32k..32k+32).
                for i in range(nt):
                    rwb = mpool.tile([128, 2, 16, 8], F32, tag="rw")
                    for h in range(2):
                        nc.vector.tensor_tensor(
                            out=rwb[:, h],
                            in0=xbs[h][:, i, :].unsqueeze(2).to_broadcast(
                                [128, 16, 8]),
                            in1=maskc_c.unsqueeze(1).to_broadcast([128, 16, 8]),
                            op=OP.mult)
                    psw = psT.tile([128, 256], F32, tag="ps_t")
                    nc.tensor.matmul(
                        out=psw[:], lhsT=s128_c,
                        rhs=rwb[:].rearrange("p h l c -> p (h l c)"),
                        start=True, stop=True)
                    i16b = ipool.tile([128, 2, 128], I16, tag="i16")
                    nc.vector.tensor_copy(out=i16b[:], in_=psw[:])
                    i16s[(t0, i)] = i16b
                w4hs[t0] = w4b

            def emit_p3(t0):
                nt = min(CHUNK, NT - t0)
                w4b = w4hs.pop(t0)
                msda = wpool.tile([128, CHUNK, 64], F32, tag="msda")
                for i in range(nt):
                    t = t0 + i
                    # g[q, h, lp, s, ch] (sample order i = lp*128 + q); the
                    # two per-head gathers fill halves of one tile so the
                    # multiply/reduce run as single double-width DVE ops.
                    g = gpool.tile([128, 2, 16, 4, 32], FP16, tag="g")
                    i16b = i16s.pop((t0, i))
                    for h in range(2):
                        nc.gpsimd.dma_gather(
                            out_ap=g[:, h].rearrange("p a b c -> p a (b c)"),
                            in_ap=tabv[h], idxs_ap=i16b[:, h, :],
                            num_idxs=2048, num_idxs_reg=ni_reg,
                            elem_size=128, single_packet=False,
                            queue_num=(t * 2 + h) % 4)
                    p16 = ppool.tile([128, 2, 16, 4, 32], FP16, tag="p16")
                    nc.vector.tensor_tensor(
                        out=p16[:], in0=g[:],
                        in1=w4b[:, i, :, :, :].unsqueeze(4).to_broadcast(
                            [128, 2, 16, 4, 32]),
                        op=OP.mult)
                    # slot sum: 3 contiguous-run adds -> ssum[q, h, lp, ch]
                    ssum = ppool.tile([128, 2, 16, 32], FP16, tag="ssum")
                    nc.vector.tensor_tensor(
                        out=ssum[:], in0=p16[:, :, :, 0, :],
                        in1=p16[:, :, :, 1, :], op=OP.add)
                    nc.vector.tensor_tensor(
                        out=ssum[:], in0=ssum[:],
                        in1=p16[:, :, :, 2, :], op=OP.add)
                    nc.vector.tensor_tensor(
                        out=ssum[:], in0=ssum[:],
                        in1=p16[:, :, :, 3, :], op=OP.add)
                    # lp sum: contiguous halving tree (fp16) -> fp32 msda
                    nc.vector.tensor_tensor(
                        out=ssum[:, :, :8, :], in0=ssum[:, :, :8, :],
                        in1=ssum[:, :, 8:, :], op=OP.add)
                    nc.vector.tensor_tensor(
                        out=ssum[:, :, :4, :], in0=ssum[:, :, :4, :],
                        in1=ssum[:, :, 4:8, :], op=OP.add)
                    nc.vector.tensor_tensor(
                        out=ssum[:, :, :2, :], in0=ssum[:, :, :2, :],
                        in1=ssum[:, :, 2:4, :], op=OP.add)
                    nc.vector.tensor_tensor(
                        out=msda[:, i, :].rearrange("p (a c) -> p a c", a=2),
                        in0=ssum[:, :, 0, :], in1=ssum[:, :, 1, :], op=OP.add)
                return msda

            def emit_p4(t0, msda):
                nt = min(CHUNK, NT - t0)
                osb = wpool.tile([128, CHUNK, 256], F32, tag="osb")
                for i in range(nt):
                    psb = psB.tile([64, 128], F32, tag="ps_b")
                    nc.tensor.transpose(out=psb[:], in_=msda[:, i, :],
                                        identity=ident[:])
                    mT = mpool.tile([64, 128], F32, tag="mT")
                    nc.scalar.copy(out=mT[:], in_=psb[:])
                    pso = psB.tile([128, 256], F32, tag="ps_o")
                    nc.tensor.matmul(out=pso[:], lhsT=mT[:], rhs=wout_sb[:],
                                     start=True, stop=True)
                    nc.scalar.copy(out=osb[:, i, :], in_=pso[:])
                nc.scalar.dma_start(
                    out=outp[t0 * 128:(t0 + nt) * 128, :].rearrange(
                        "(t q) c -> q t c", q=128),
                    in_=osb[:, :nt, :])

            emit_p2(0)
            # ---- P1: value projection -> vtab ----
            def emit_p1b(lv):
                lh, lw = SPATIAL[lv]
                s0, hw = STARTS[lv], HWS[lv]
                for h in range(2):
                    for s, sh in enumerate((0, 1, lw, lw + 1)):
                        nc.sync.dma_start(
                            out=tab[h][s0:s0 + hw, s * 32:(s + 1) * 32],
                            in_=vtab[h][s0 + sh:s0 + sh + hw, :])

            t0 = 0
            while t0 < NT:
                nt = min(CHUNK, NT - t0)
                vas = []
                for i, (o, kk) in enumerate(KV_CH):
                    va = spool.tile([128, CHUNK * 128], BF16, tag="vld",
                                    name=f"vab{i}")
                    nc.sync.dma_start(
                        out=va[:kk, :nt * 128],
                        in_=vaug[o:o + kk, t0 * 128:(t0 + nt) * 128])
                    vas.append(va)
                vsb = mpool.tile([128, CHUNK, 64], FP16, tag="vsb")
                for i in range(nt):
                    psv = psA.tile([128, 64], F32, tag="ps_a")
                    for j, (o, kk) in enumerate(KV_CH):
                        nc.tensor.matmul(
                            out=psv[:], lhsT=vas[j][:kk, i * 128:(i + 1) * 128],
                            rhs=wv_sb[j][:],
                            start=(j == 0), stop=(j == len(KV_CH) - 1))
                    nc.scalar.copy(out=vsb[:, i, :], in_=psv[:])
                for h in range(2):
                    nc.sync.dma_start(
                        out=vtab[h][t0 * 128:(t0 + nt) * 128, :].rearrange(
                            "(t q) c -> q t c", q=128),
                        in_=vsb[:, :nt, h * 32:(h + 1) * 32])
                t0 += nt
                # interleave patch-table builds: emit each level's slot DMAs
                # as soon as the vtab rows it reads are stored
                if t0 == 72:
                    emit_p1b(0)
                elif t0 == 88:
                    emit_p1b(1)
            emit_p1b(2)
            emit_p1b(3)

            chunk_starts = list(range(0, NT, CHUNK))
            for ci, t0 in enumerate(chunk_starts):
                if ci + 1 < len(chunk_starts):
                    emit_p2(chunk_starts[ci + 1])
                msda = emit_p3(t0)
                emit_p4(t0, msda)
    nc.compile()
    return nc


def host_prep(inputs):
    q = np.asarray(inputs["query"], np.float32)
    ref = np.asarray(inputs["reference_points"], np.float32)
    val = np.asarray(inputs["value"], np.float32)
    W_off = np.asarray(inputs["W_off"], np.float32)
    b_off = np.asarray(inputs["b_off"], np.float32)
    W_attn = np.asarray(inputs["W_attn"], np.float32)
    b_attn = np.asarray(inputs["b_attn"], np.float32)
    W_val = np.asarray(inputs["W_val"], np.float32)
    b_val = np.asarray(inputs["b_val"], np.float32)
    W_out = np.asarray(inputs["W_out"], np.float32)

    lh = np.array([s[0] for s in SPATIAL], np.float32)
    lw = np.array([s[1] for s in SPATIAL], np.float32)

    qaug = np.zeros((B, KA, NQP), np.float32)
    for b in range(B):
        qaug[b, :DM, :NQ] = q[b].T
        rx = ref[b, :, :, 0] * lw[None, :] - 1.0   # px' = px - 0.5
        ry = ref[b, :, :, 1] * lh[None, :] - 1.0
        qaug[b, DM:DM + 16, :NQ] = np.repeat(rx, P, axis=1).T
        qaug[b, DM + 16:DM + 32, :NQ] = np.repeat(ry, P, axis=1).T
        qaug[b, DM + 32, :] = 1.0

    vaug = np.zeros((B, KV, NQP), np.float32)
    for b in range(B):
        vaug[b, :DM, :NV] = val[b].T
        vaug[b, DM, :] = 1.0

    W_off_r = W_off.reshape(DM, H, L, P, 2)
    b_off_r = b_off.reshape(H, L, P, 2)
    W_attn_r = W_attn.reshape(DM, H, L, P)
    b_attn_r = b_attn.reshape(H, L, P)

    waug_all = np.zeros((H, KA, 48), np.float32)
    for h in range(H):
        waug_all[h, :DM, 0:16] = W_off_r[:, h, :, :, 0].reshape(DM, 16)
        waug_all[h, :DM, 16:32] = W_off_r[:, h, :, :, 1].reshape(DM, 16)
        waug_all[h, :DM, 32:48] = W_attn_r[:, h].reshape(DM, 16)
        waug_all[h, DM + 32, 0:16] = b_off_r[h, :, :, 0].reshape(16)
        waug_all[h, DM + 32, 16:32] = b_off_r[h, :, :, 1].reshape(16)
        waug_all[h, DM + 32, 32:48] = b_attn_r[h].reshape(16)
        for j in range(16):
            waug_all[h, DM + j, j] = 1.0
            waug_all[h, DM + 16 + j, 16 + j] = 1.0

    W_val_r = W_val.reshape(DM, H, HD)
    b_val_r = b_val.reshape(H, HD)

    consts = np.zeros((128, 6, 16), np.float32)
    consts[:, 0, :] = np.repeat(lw, P)[None, :]
    consts[:, 1, :] = np.repeat(lw - 1.0, P)[None, :]
    consts[:, 2, :] = np.repeat(lh - 1.0, P)[None, :]
    consts[:, 3, :] = np.repeat(np.array(STARTS, np.float32), P)[None, :]
    consts[:, 4, :] = np.repeat(lw - 1.5, P)[None, :]
    consts[:, 5, :] = np.repeat(lh - 1.5, P)[None, :]

    wrapc = np.zeros((128, 136), np.float32)
    for q in range(128):
        for m in range(128):
            if q % 16 == m % 16:
                wrapc[q, m] = 1.0
        wrapc[q, 128 + q // 16] = 1.0

    in_maps = []
    for c in range(N_CORES):
        b = c // 4
        h0 = 2 * (c % 4)
        waug = np.concatenate([waug_all[h0], waug_all[h0 + 1]], axis=1)
        wv = np.zeros((KV, 64), np.float32)
        wv[:DM, 0:32] = W_val_r[:, h0, :]
        wv[:DM, 32:64] = W_val_r[:, h0 + 1, :]
        wv[DM, 0:32] = b_val_r[h0]
        wv[DM, 32:64] = b_val_r[h0 + 1]
        wo = np.ascontiguousarray(
            W_out.reshape(H, HD, DM)[h0:h0 + 2].reshape(64, DM))
        in_maps.append({
            "qaug": np.ascontiguousarray(qaug[b]),
            "vaug": np.ascontiguousarray(vaug[b]).astype(ml_dtypes.bfloat16),
            "waug": np.ascontiguousarray(waug),
            "wvaug": wv.astype(ml_dtypes.bfloat16),
            "wout": wo,
            "consts": consts,
            "wrapc": wrapc,
        })
    return in_maps


_NC_CACHE = None


def kernel(**inputs) -> np.ndarray:
    global _NC_CACHE
    in_maps = host_prep(inputs)
    if _NC_CACHE is None:
        _NC_CACHE = build_module()
    nc = _NC_CACHE
    res = run_bass_kernel_spmd(nc, in_maps, core_ids=list(range(N_CORES)))
    b_out = np.asarray(inputs["b_out"], np.float32)
    out = np.zeros((B, NQ, DM), np.float32)
    for c in range(N_CORES):
        out[c // 4] += res.results[c]["outp"][:NQ, :]
    out += b_out[None, None, :]
    return out


if __name__ == "__main__":
    import reference

    inputs = {k: np.asarray(v) for k, v in reference.setup_inputs().items()}
    got = kernel(**inputs)
    exp = np.asarray(reference.reference(**inputs))
    err = np.abs(got - exp)
    rel = np.linalg.norm(got - exp) / np.linalg.norm(exp)
    print("abs max err:", err.max(), "rel:", rel)

